# revision 9
# baseline (speedup 1.0000x reference)
"""Trainium2 Bass kernel for nn_BPBookMemory (retrieval_knn).

Strategy (8 NeuronCores, SPMD):
  - x sharded by batch (8 per core); memory bank sharded 8-way (8192 rows/core).
  - Warmup collective triggered at t=0 (no input DMA) so the ~60us cold-start
    of the collectives subsystem overlaps Phase A instead of serializing.
  - Phase A: stream x (p-outer layout: each partition owns a contiguous
    32-token block -> 16KB DMA lines), cast to bf16 on GpSimd, PE-transpose,
    featT = gelu(W xT + b), accumulate q sums per batch on ACT (accum_out).
  - Phase B (interleaved with A in emission order so it overlaps): load
    memory shard, bf16 raw copy (GpSimd), row norms (ACT square+accum),
    normalize (DVE), PE-transpose -> mt tiles.
  - AllGather q -> all 64 query vectors everywhere.
  - sim[b, s_local] matmuls for all 64 batches; block-wise max8 gives 64
    candidate values per batch per core.
  - AllGather candidates -> identical merge on every core via max8 +
    match_replace + max8 -> global top-16 values, threshold, softmax scalars.
  - Dense masked softmax weights W = mask * exp(...) in bf16, PE-transpose,
    partial proto = W @ memory_shard; ReduceScatter(add).
  - out = x + retrieval_scale * proto, stored as bf16 (upcast to f32 on host;
    bf16 rounding of the output is ~0.2% rel, far under the 2e-2 gate).

Index-free top-k: only candidate VALUES travel; selection is by threshold
(sim >= 16th-largest), so no max_index / gather is ever needed.
"""

import os
import sys

for _p in ("/opt/trn_rl_repo", "/root/.axon_site/_ro/trn_rl_repo"):
    if os.path.isdir(_p) and _p not in sys.path:
        sys.path.append(_p)

import numpy as np
from contextlib import ExitStack

import concourse.bass as bass
import concourse.tile as tile
from concourse import mybir
from concourse.bass_utils import run_bass_kernel_spmd
from concourse.vector_clock import ScopedClock

F32 = mybir.dt.float32
BF16 = mybir.dt.bfloat16
AF = mybir.ActivationFunctionType
ALU = mybir.AluOpType

NCORES = 8
B, N, D, S = 64, 4096, 128, 65536
BL = B // NCORES          # 8 batches per core
SL = S // NCORES          # 8192 memory rows per core
MT = SL // 128            # 64 memory tiles per core
MC = SL // 512            # 16 memory chunks of 512
NEG_BIG = -1.0e30


# ---------------------------------------------------------------------------
# Walrus workaround: this container's neuronxcc rejects instructions carrying
# more than ~1 sync wait command (Drain/TPB_CTRL, LDWEIGHTS/S3_LW...).
# 1) Replace Tile's exit drain+barrier with EventSemaphore-carried waits.
# 2) Post-pass: hoist excess waits onto standalone EventSemaphore insts.
# ---------------------------------------------------------------------------

def _patched_drain_and_barrier(self, tick_clock, wait_clock):
    nc = self.nc
    carrier = nc.sync.add_instruction(
        mybir.InstEventSemaphore(name=f"I-{nc.next_id()}", ins=[], outs=[])
    )
    wait_clock.add_sem_waits(carrier.ins, ScopedClock({None: tick_clock.global_clock}))
    si = carrier.ins.sync_info
    waits = list(si.on_wait or [])
    if len(waits) > 1:
        si.on_wait = [waits[0]]
        for w in waits[1:]:
            extra = nc.sync.add_instruction(
                mybir.InstEventSemaphore(name=f"I-{nc.next_id()}", ins=[], outs=[])
            )
            extra.ins.sync_info = mybir.SyncInfo(on_wait=[w], on_update=[])
    for eng in nc.engines.values():
        eng.drain()
    nc.all_engine_barrier(sem_only=True)
    popped = nc._tile_sem_poison_stack.pop()
    assert popped is self._sem_poison
    nc.clear_and_free_semaphores(list(self.sems.allocated().values()))
    nc.all_engine_barrier(sem_only=True)


tile.TileContext._drain_and_barrier = _patched_drain_and_barrier

_hoist_ctr = [0]


def _hoist_waits(nc, max_keep=1):
    for f in nc.m.functions:
        for bb in f.blocks:
            insts = bb.instructions
            out = []
            changed = False
            for inst in insts:
                si = inst.sync_info
                waits = list(si.on_wait) if (si is not None and si.on_wait) else []
                if waits:
                    keep = 0 if inst.opcode == "Drain" else max_keep
                    kept, hoisted = [], []
                    for w in waits:
                        if len(kept) < keep and w.wait_mode == "sem-ge-imm":
                            kept.append(w)
                        else:
                            hoisted.append(w)
                    if hoisted:
                        for w in hoisted:
                            _hoist_ctr[0] += 1
                            ev = mybir.InstEventSemaphore(
                                name=f"I-hoistw-{_hoist_ctr[0]}", ins=[], outs=[]
                            )
                            ev.engine = inst.engine
                            ev.sync_info = mybir.SyncInfo(on_wait=[w], on_update=[])
                            out.append(ev)
                        si.on_wait = kept
                        changed = True
                out.append(inst)
            if changed:
                bb.instructions = out


# ---------------------------------------------------------------------------
# Kernel build
# ---------------------------------------------------------------------------

def build_program(debug=False):
    nc = bass.Bass(num_devices=NCORES)
    groups = [list(range(NCORES))]

    # raise Tile's stale SBUF cap (cayman has 208 KB usable per partition)
    import concourse.tile_utils as tile_utils
    if getattr(tile_utils, "max_sbuf_usage", 0) < 200 * 1024:
        tile_utils.max_sbuf_usage = 200 * 1024

    xs = nc.dram_tensor("xs", [BL, N, D], F32, kind="ExternalInput")
    ms = nc.dram_tensor("ms", [SL, D], F32, kind="ExternalInput")
    convw = nc.dram_tensor("convw", [D, D], F32, kind="ExternalInput")
    convb = nc.dram_tensor("convb", [D], F32, kind="ExternalInput")
    scal = nc.dram_tensor("scal", [1], F32, kind="ExternalInput")
    identb_in = nc.dram_tensor("identb", [128, 128], BF16, kind="ExternalInput")
    out_ext = nc.dram_tensor("out", [BL, N, D], BF16, kind="ExternalOutput")

    # collective bounce buffers
    warm_in = nc.dram_tensor("warm_in", [8, 4], F32)
    warm_out = nc.dram_tensor("warm_out", [8, 4], F32, addr_space="Shared")
    q_in = nc.dram_tensor("q_in", [128, BL], F32)
    q_ag = nc.dram_tensor("q_ag", [128 * NCORES, BL], F32, addr_space="Shared")
    cand_in = nc.dram_tensor("cand_in", [B, 64], F32)
    cand_ag = nc.dram_tensor("cand_ag", [B * NCORES, 64], F32, addr_space="Shared")
    proto_in = nc.dram_tensor("proto_in", [B, D], F32)
    proto_rs = nc.dram_tensor("proto_rs", [BL, D], F32)

    with tile.TileContext(nc) as tc, ExitStack() as top:
        # warmup collective FIRST: no input DMA (contents unused), so the
        # trigger has no dependencies and fires at t~0, absorbing the
        # collectives-subsystem cold start under Phase A.
        nc.gpsimd.collective_compute(
            "AllReduce", ALU.add, replica_groups=groups,
            ins=[warm_in[:]], outs=[warm_out[:]],
        )

        cst = top.enter_context(tc.tile_pool(name="cst", bufs=1))
        big = top.enter_context(tc.tile_pool(name="big", bufs=1))
        sml = top.enter_context(tc.tile_pool(name="sml", bufs=1))

        # constants on the scalar HWDGE ring so the sync ring starts x
        # loads immediately.
        identb = cst.tile([128, 128], BF16)
        nc.scalar.dma_start(identb[:], identb_in[:])
        ones = cst.tile([128, 128], F32)
        nc.gpsimd.memset(ones[:], 1.0)
        zeros = cst.tile([128, 1], F32)
        nc.gpsimd.memset(zeros[:], 0.0)
        bias_col = cst.tile([128, 1], F32)
        nc.scalar.dma_start(bias_col[:], convb[:].rearrange("(p o) -> p o", o=1))
        scal_sb = cst.tile([1, 1], F32)
        nc.scalar.dma_start(scal_sb[:], scal[:].rearrange("(p o) -> p o", o=1))

        # conv_w -> WT bf16 in SBUF (cast + single bf16 PE transpose)
        wconv = cst.tile([128, 128], F32)
        nc.scalar.dma_start(wconv[:], convw[:])
        wconv_b = cst.tile([128, 128], BF16)
        nc.vector.tensor_copy(wconv_b[:], wconv[:])
        wt_conv = cst.tile([128, 128], BF16)

        # persistent SBUF
        xb = [big.tile([128, N], BF16, name=f"xb{b}", tag=f"xb{b}")
              for b in range(BL)]                      # 8 KB/part each
        sim_sb = big.tile([128, 4096], BF16)           # 8 KB/part (fold-2)
        mraw = big.tile([128, SL], BF16)               # raw memory bf16, 16 KB/part
        wb_t = big.tile([128, 4096], BF16)             # masked softmax W, 8 KB/part
        mtsb = big.tile([128, SL], BF16)               # normalized memory^T, 16 KB/part
        qacc = sml.tile([128, 32], F32)
        qT_all = sml.tile([128, B], F32)
        qTb = sml.tile([128, B], BF16)
        cands = sml.tile([128, 32], F32)
        cand_all = sml.tile([B, NCORES * 64], F32)
        mr_scr = sml.tile([B, NCORES * 64], F32)
        t16 = sml.tile([B, 16], F32)
        e16 = sml.tile([B, 16], F32)
        params = sml.tile([128, 4], F32)
        ssq = sml.tile([128, MT], F32)
        minv = sml.tile([128, MT], F32)
        proto_sb = sml.tile([B, D], F32)
        proto_loc = sml.tile([1, BL * D], F32)
        cwork = sml.tile([64, 8], F32)

        scal_col = cst.tile([128, 1], F32)
        with tc.tile_pool(name="wt0ps", bufs=1, space="PSUM") as wt0ps:
            wtp = wt0ps.tile([128, 128], BF16)
            nc.tensor.transpose(wtp[:], wconv_b[:], identb[:])
            nc.vector.tensor_copy(wt_conv[:], wtp[:])
            scp = wt0ps.tile([128, 1], F32)
            nc.tensor.matmul(scp[:], ones[0:1, :], scal_sb[0:1, 0:1],
                             start=True, stop=True)
            nc.vector.tensor_copy(scal_col[:], scp[:])

        # ---- Phases A+B interleaved -------------------------------------
        with ExitStack() as pa:
            xstp = pa.enter_context(tc.tile_pool(name="xst", bufs=2))
            xt_sbp = pa.enter_context(tc.tile_pool(name="xt_sb", bufs=4))
            gelp = pa.enter_context(tc.tile_pool(name="gel", bufs=2))
            xt_ps = pa.enter_context(tc.tile_pool(name="xt_ps", bufs=3, space="PSUM"))
            ft_ps = pa.enter_context(tc.tile_pool(name="ft_ps", bufs=2, space="PSUM"))
            m_in = pa.enter_context(tc.tile_pool(name="m_in", bufs=2))
            mn_p = pa.enter_context(tc.tile_pool(name="mn", bufs=2))
            sq_p = pa.enter_context(tc.tile_pool(name="sq", bufs=2))

            def emit_b_load(c):
                # memory chunk c (1024 rows): load + bf16 raw copy + norm
                # chain (ACT square -> DVE grouped reduce -> sqrt -> recip ->
                # one DVE broadcast multiply).  Transposes happen one round
                # later (emit_b_transpose) so the PE never waits on this
                # chain.
                mi = m_in.tile([128, 1024], F32, name="mi", tag="mi")
                nc.sync.dma_start(
                    mi[:].rearrange("p (t d) -> p t d", d=128),
                    ms[c * 1024:(c + 1) * 1024].rearrange("(t p) d -> p t d",
                                                          p=128),
                )
                nc.vector.tensor_copy(mraw[:, c * 1024:(c + 1) * 1024], mi[:])
                sq = sq_p.tile([128, 1024], BF16, name="sq", tag="sq")
                nc.scalar.activation(sq[:], mi[:], AF.Square, bias=zeros[:])
                iv = minv[:, c * 8:c * 8 + 8]
                nc.vector.tensor_reduce(
                    ssq[:, c * 8:c * 8 + 8],
                    sq[:].rearrange("p (t d) -> p t d", d=128),
                    axis=mybir.AxisListType.X, op=ALU.add,
                )
                nc.scalar.activation(iv, ssq[:, c * 8:c * 8 + 8], AF.Sqrt,
                                     bias=zeros[:])
                nc.vector.reciprocal(iv, iv)
                mn = mn_p.tile([128, 1024], BF16, name="mn", tag="mn")
                nc.vector.tensor_tensor(
                    mn[:].rearrange("p (t d) -> p t d", d=128),
                    mi[:].rearrange("p (t d) -> p t d", d=128),
                    iv.rearrange("p (t o) -> p t o", o=1).broadcast_to(
                        [128, 8, 128]),
                    op=ALU.mult,
                )
                return mn

            def emit_b_transpose(c, mn):
                mp = xt_ps.tile([128, 1024], BF16, name="xp", tag="xp")
                for k in range(8):
                    nc.tensor.transpose(
                        mp[:, k * 128:(k + 1) * 128],
                        mn[:, k * 128:(k + 1) * 128], identb[:],
                    )
                nc.vector.tensor_copy(mtsb[:, c * 1024:(c + 1) * 1024], mp[:])

            def emit_a_batch(b):
                # batch b: load [128, 4096] f32 (p-outer: partition p owns
                # tokens p*32..p*32+31 -> contiguous 16KB DMA lines), cast to
                # bf16 (DVE, all 4 groups up front so the PE never waits on a
                # cast stuck behind PSUM copies), PE transpose, feat matmul,
                # gelu+accum (ACT).
                xstage = xstp.tile([128, N], F32)
                if b == 0:
                    # split the first load so compute starts ~4x earlier
                    for j in range(4):
                        nc.sync.dma_start(
                            xstage[:, j * 1024:(j + 1) * 1024].rearrange(
                                "p (t d) -> p t d", d=128),
                            xs[b].rearrange("(p t) d -> p t d", p=128)[
                                :, j * 8:(j + 1) * 8, :],
                        )
                else:
                    nc.sync.dma_start(
                        xstage[:].rearrange("p (t d) -> p t d", d=128),
                        xs[b].rearrange("(p t) d -> p t d", p=128),
                    )
                for j in range(4):
                    nc.vector.tensor_copy(
                        xb[b][:, j * 1024:(j + 1) * 1024],
                        xstage[:, j * 1024:(j + 1) * 1024],
                    )
                for j in range(4):          # 1024-col groups
                    base = j * 1024
                    xp = xt_ps.tile([128, 1024], BF16, name="xp", tag="xp")
                    for k in range(8):
                        nc.tensor.transpose(
                            xp[:, k * 128:(k + 1) * 128],
                            xb[b][:, base + k * 128:base + (k + 1) * 128],
                            identb[:],
                        )
                    xsb = xt_sbp.tile([128, 1024], BF16)
                    if j % 2 == 0:
                        nc.vector.tensor_copy(xsb[:], xp[:])
                    else:
                        nc.scalar.copy(xsb[:], xp[:])
                    fp = ft_ps.tile([128, 1024], F32)
                    nc.tensor.matmul(fp[:, 0:512], wt_conv[:], xsb[:, 0:512],
                                     start=True, stop=True)
                    nc.tensor.matmul(fp[:, 512:1024], wt_conv[:],
                                     xsb[:, 512:1024], start=True, stop=True)
                    gl = gelp.tile([128, 1024], BF16, name="gl", tag="gl")
                    col = 4 * b + j
                    nc.scalar.activation(
                        gl[:], fp[:], AF.Gelu,
                        bias=bias_col[:], accum_out=qacc[:, col:col + 1],
                    )

            mn_prev = None
            for b in range(BL):
                emit_a_batch(b)
                if mn_prev is not None:
                    emit_b_transpose(b - 1, mn_prev)
                mn_prev = emit_b_load(b)
            emit_b_transpose(BL - 1, mn_prev)

            qT = sml.tile([128, BL], F32)
            nc.vector.tensor_reduce(
                qT[:], qacc[:].rearrange("p (b g) -> p b g", g=4),
                axis=mybir.AxisListType.X, op=ALU.add,
            )
            nc.sync.dma_start(q_in[:], qT[:])

        nc.gpsimd.collective_compute(
            "AllGather", ALU.bypass, replica_groups=groups,
            ins=[q_in[:]], outs=[q_ag[:]],
        )
        # keep the PE busy (and the HAM clock-gate open) while the q
        # AllGather is in flight: dummy transposes with no consumers.
        with tc.tile_pool(name="warm_ps", bufs=1, space="PSUM") as wps_pool:
            wrm = wps_pool.tile([128, 1024], BF16)
            for i in range(56):
                nc.tensor.transpose(
                    wrm[:, (i % 8) * 128:(i % 8 + 1) * 128],
                    mtsb[:, (i % 8) * 128:(i % 8 + 1) * 128], identb[:],
                )
        nc.sync.dma_start(
            qT_all[:].rearrange("p (c b) -> p c b", c=NCORES),
            q_ag[:].rearrange("(c p) b -> p c b", p=128),
        )
        nc.vector.tensor_copy(qTb[:], qT_all[:])

        # cinv = 1/||q_b||
        qsq = sml.tile([128, B], F32)
        nc.vector.tensor_tensor(qsq[:], qT_all[:], qT_all[:], op=ALU.mult)
        with tc.tile_pool(name="nrm_ps", bufs=1, space="PSUM") as nrmp:
            nrm = nrmp.tile([1, B], F32)
            nc.tensor.matmul(nrm[:], ones[:, 0:1], qsq[:], start=True, stop=True)
            nrow = sml.tile([1, B], F32)
            nc.scalar.activation(nrow[:], nrm[:], AF.Sqrt, bias=zeros[0:1, :])
            nc.vector.reciprocal(nrow[:], nrow[:])
            ncol = nrmp.tile([B, 1], F32)
            nc.tensor.matmul(ncol[:], nrow[:], ones[0:1, 0:1],
                             start=True, stop=True)
            nc.vector.tensor_copy(params[0:B, 0:1], ncol[:])

        # ---- sim matmuls (fold-2 into 128-part psum tiles) ---------------
        with tc.tile_pool(name="sim_ps", bufs=2, space="PSUM") as sim_ps:
            for cc in range(MC // 2):
                sp = sim_ps.tile([128, 512], F32)
                for half in range(2):
                    c = half * (MC // 2) + cc
                    nc.tensor.matmul(sp[half * 64:half * 64 + 64, :],
                                     qTb[:], mtsb[:, c * 512:(c + 1) * 512],
                                     start=True, stop=True)
                nc.vector.tensor_copy(sim_sb[:, cc * 512:(cc + 1) * 512], sp[:])

        for blk in range(4):
            nc.vector.max(
                cands[:, blk * 8:(blk + 1) * 8],
                sim_sb[:, blk * 1024:(blk + 1) * 1024],
            )
        nc.sync.dma_start(cand_in[:, 0:32], cands[0:64, :])
        nc.sync.dma_start(cand_in[:, 32:64], cands[64:128, :])

        nc.gpsimd.collective_compute(
            "AllGather", ALU.bypass, replica_groups=groups,
            ins=[cand_in[:]], outs=[cand_ag[:]],
        )
        nc.sync.dma_start(
            cand_all[:].rearrange("b (c j) -> b c j", c=NCORES),
            cand_ag[:].rearrange("(c b) j -> b c j", b=B),
        )

        # ---- merge: global top-16, softmax scalars -----------------------
        nc.vector.max(t16[:, 0:8], cand_all[:])
        nc.vector.match_replace(mr_scr[:], t16[:, 0:8], cand_all[:], NEG_BIG)
        nc.vector.max(t16[:, 8:16], mr_scr[:])

        nc.vector.tensor_tensor(cwork[:, 0:1], t16[:, 0:1], params[0:B, 0:1],
                                op=ALU.mult)
        nc.vector.tensor_scalar_mul(cwork[:, 1:2], cwork[:, 0:1], -1.0)
        nc.scalar.activation(e16[:], t16[:], AF.Exp,
                             bias=cwork[:, 1:2], scale=params[0:B, 0:1])
        nc.vector.tensor_reduce(cwork[:, 2:3], e16[:],
                                axis=mybir.AxisListType.X, op=ALU.add)
        nc.scalar.activation(cwork[:, 3:4], cwork[:, 2:3], AF.Ln,
                             bias=zeros[0:B, :])
        nc.vector.tensor_tensor(params[0:B, 1:2], cwork[:, 1:2], cwork[:, 3:4],
                                op=ALU.subtract)
        nc.vector.tensor_copy(params[0:B, 2:3], t16[:, 15:16])
        nc.sync.dma_start(params[64:128, 0:3], params[0:64, 0:3])

        if debug:
            dbg_t16 = nc.dram_tensor("dbg_t16", [B, 16], F32,
                                     kind="ExternalOutput")
            dbg_params = nc.dram_tensor("dbg_params", [128, 4], F32,
                                        kind="ExternalOutput")
            dbg_proto = nc.dram_tensor("dbg_proto", [B, D], F32,
                                       kind="ExternalOutput")
            nc.sync.dma_start(dbg_t16[:], t16[:])
            nc.sync.dma_start(dbg_params[:], params[:])
            nc.sync.dma_start(dbg_proto[:], proto_sb[:])

        # ---- Phase D: dense masked softmax W -> partial proto ------------
        with ExitStack() as pd:
            maskp = pd.enter_context(tc.tile_pool(name="mask", bufs=2))
            wt_sbp = pd.enter_context(tc.tile_pool(name="wt_sb", bufs=2))
            wt_psp = pd.enter_context(tc.tile_pool(name="wt_ps", bufs=2, space="PSUM"))
            pr_ps = pd.enter_context(tc.tile_pool(name="pr_ps", bufs=1, space="PSUM"))

            for quar in range(4):
                qs = slice(quar * 1024, (quar + 1) * 1024)
                mk = maskp.tile([128, 1024], BF16)
                nc.vector.tensor_scalar(
                    mk[:], sim_sb[:, qs], params[:, 2:3], None, op0=ALU.is_ge
                )
                nc.scalar.activation(
                    wb_t[:, qs], sim_sb[:, qs], AF.Exp,
                    bias=params[:, 1:2], scale=params[:, 0:1],
                )
                nc.vector.tensor_tensor(
                    wb_t[:, qs], wb_t[:, qs], mk[:], op=ALU.mult
                )

            pr = pr_ps.tile([64, 128], F32)
            for half in range(2):
                for k0 in range(0, 32, 8):
                    idh = identb[half * 64:half * 64 + 64,
                                 half * 64:half * 64 + 64]
                    wps = wt_psp.tile([128, 512], BF16)
                    for kk in range(8):
                        k = k0 + kk
                        nc.tensor.transpose(
                            wps[:, kk * 64:(kk + 1) * 64],
                            wb_t[half * 64:half * 64 + 64,
                                 k * 128:(k + 1) * 128],
                            idh,
                        )
                    wsb = wt_sbp.tile([128, 512], BF16)
                    nc.vector.tensor_copy(wsb[:], wps[:])
                    for kk in range(8):
                        t = half * 32 + k0 + kk
                        nc.tensor.matmul(
                            pr[:], wsb[:, kk * 64:(kk + 1) * 64],
                            mraw[:, t * 128:(t + 1) * 128],
                            start=(t == 0), stop=(t == MT - 1),
                        )
            nc.vector.tensor_copy(proto_sb[:], pr[:])
            nc.sync.dma_start(proto_in[:], proto_sb[:])

        nc.gpsimd.collective_compute(
            "ReduceScatter", ALU.add, replica_groups=groups,
            ins=[proto_in[:]], outs=[proto_rs[:]],
        )
        nc.sync.dma_start(proto_loc[:], proto_rs[:].rearrange("b d -> (b d)")
                          .rearrange("(o f) -> o f", o=1))

        # ---- Phase E: out = x + scale * proto broadcast (bf16) -----------
        with tc.tile_pool(name="bb_ps", bufs=2, space="PSUM") as bbp, \
             tc.tile_pool(name="bb_sb", bufs=2) as bbs:
            for b in range(BL):
                pb_ = bbp.tile([128, 128], F32)
                nc.tensor.matmul(pb_[:], ones[0:1, :],
                                 proto_loc[0:1, b * 128:(b + 1) * 128],
                                 start=True, stop=True)
                pbs = bbs.tile([128, 128], BF16)
                nc.vector.tensor_scalar(pbs[:], pb_[:], scal_col[:, 0:1],
                                        None, op0=ALU.mult)
                seg = xb[b][:].rearrange("p (t d) -> p t d", d=128)
                nc.vector.tensor_tensor(
                    seg, seg,
                    pbs[:].rearrange("p (o d) -> p o d", o=1).broadcast_to(
                        [128, N // 128, 128]
                    ),
                    op=ALU.add,
                )
                nc.sync.dma_start(
                    out_ext[b].rearrange("(p t) d -> p t d", p=128),
                    seg,
                )

    _hoist_waits(nc)
    return nc


_CACHED = {}


def kernel(x, conv_w, conv_b, memory, retrieval_scale):
    import ml_dtypes
    x = np.ascontiguousarray(np.asarray(x, dtype=np.float32))
    conv_w = np.ascontiguousarray(np.asarray(conv_w, dtype=np.float32))
    conv_b = np.ascontiguousarray(np.asarray(conv_b, dtype=np.float32))
    memory = np.ascontiguousarray(np.asarray(memory, dtype=np.float32))
    scal = np.asarray(retrieval_scale, dtype=np.float32).reshape(1)
    identb = np.eye(128, dtype=ml_dtypes.bfloat16)

    if "nc" not in _CACHED:
        _CACHED["nc"] = build_program()
    nc = _CACHED["nc"]

    in_maps = []
    for c in range(NCORES):
        in_maps.append({
            "xs": x[c * BL:(c + 1) * BL],
            "ms": memory[c * SL:(c + 1) * SL],
            "convw": conv_w,
            "convb": conv_b,
            "scal": scal,
            "identb": identb,
        })
    res = run_bass_kernel_spmd(nc, in_maps, list(range(NCORES)),
                               **_CACHED.get("run_kwargs", {}))
    _CACHED["last_result"] = res
    out = np.empty_like(x)
    for c in range(NCORES):
        out[c * BL:(c + 1) * BL] = np.asarray(res.results[c]["out"],
                                              dtype=np.float32)
    return out


# revision 12
# speedup vs baseline: 1.1130x; 1.1130x over previous
"""Trainium2 Bass kernel for nn_BPBookMemory (retrieval_knn).

Strategy (8 NeuronCores, SPMD):
  - x sharded by batch (8 per core); memory bank sharded 8-way (8192 rows/core).
  - Warmup collective triggered at t=0 (no input DMA) so the ~60us cold-start
    of the collectives subsystem overlaps Phase A instead of serializing.
  - Phase A: stream x (p-outer layout: each partition owns a contiguous
    32-token block -> 16KB DMA lines), cast to bf16 on GpSimd, PE-transpose,
    featT = gelu(W xT + b), accumulate q sums per batch on ACT (accum_out).
  - Phase B (interleaved with A in emission order so it overlaps): load
    memory shard, bf16 raw copy (GpSimd), row norms (ACT square+accum),
    normalize (DVE), PE-transpose -> mt tiles.
  - AllGather q -> all 64 query vectors everywhere.
  - sim[b, s_local] matmuls for all 64 batches; block-wise max8 gives 64
    candidate values per batch per core.
  - AllGather candidates -> identical merge on every core via max8 +
    match_replace + max8 -> global top-16 values, threshold, softmax scalars.
  - Dense masked softmax weights W = mask * exp(...) in bf16, PE-transpose,
    partial proto = W @ memory_shard; ReduceScatter(add).
  - out = x + retrieval_scale * proto, stored as bf16 (upcast to f32 on host;
    bf16 rounding of the output is ~0.2% rel, far under the 2e-2 gate).

Index-free top-k: only candidate VALUES travel; selection is by threshold
(sim >= 16th-largest), so no max_index / gather is ever needed.
"""

import os
import sys

for _p in ("/opt/trn_rl_repo", "/root/.axon_site/_ro/trn_rl_repo"):
    if os.path.isdir(_p) and _p not in sys.path:
        sys.path.append(_p)

import numpy as np
from contextlib import ExitStack

import concourse.bass as bass
import concourse.tile as tile
from concourse import mybir
from concourse.bass_utils import run_bass_kernel_spmd
from concourse.vector_clock import ScopedClock

F32 = mybir.dt.float32
BF16 = mybir.dt.bfloat16
AF = mybir.ActivationFunctionType
ALU = mybir.AluOpType

NCORES = 8
B, N, D, S = 64, 4096, 128, 65536
BL = B // NCORES          # 8 batches per core
SL = S // NCORES          # 8192 memory rows per core
MT = SL // 128            # 64 memory tiles per core
MC = SL // 512            # 16 memory chunks of 512
NEG_BIG = -1.0e30


# ---------------------------------------------------------------------------
# Walrus workaround: this container's neuronxcc rejects instructions carrying
# more than ~1 sync wait command (Drain/TPB_CTRL, LDWEIGHTS/S3_LW...).
# 1) Replace Tile's exit drain+barrier with EventSemaphore-carried waits.
# 2) Post-pass: hoist excess waits onto standalone EventSemaphore insts.
# ---------------------------------------------------------------------------

def _patched_drain_and_barrier(self, tick_clock, wait_clock):
    nc = self.nc
    carrier = nc.sync.add_instruction(
        mybir.InstEventSemaphore(name=f"I-{nc.next_id()}", ins=[], outs=[])
    )
    wait_clock.add_sem_waits(carrier.ins, ScopedClock({None: tick_clock.global_clock}))
    si = carrier.ins.sync_info
    waits = list(si.on_wait or [])
    if len(waits) > 1:
        si.on_wait = [waits[0]]
        for w in waits[1:]:
            extra = nc.sync.add_instruction(
                mybir.InstEventSemaphore(name=f"I-{nc.next_id()}", ins=[], outs=[])
            )
            extra.ins.sync_info = mybir.SyncInfo(on_wait=[w], on_update=[])
    for eng in nc.engines.values():
        eng.drain()
    nc.all_engine_barrier(sem_only=True)
    popped = nc._tile_sem_poison_stack.pop()
    assert popped is self._sem_poison
    nc.clear_and_free_semaphores(list(self.sems.allocated().values()))
    nc.all_engine_barrier(sem_only=True)


tile.TileContext._drain_and_barrier = _patched_drain_and_barrier

_hoist_ctr = [0]


def _hoist_waits(nc, max_keep=1):
    for f in nc.m.functions:
        for bb in f.blocks:
            insts = bb.instructions
            out = []
            changed = False
            for inst in insts:
                si = inst.sync_info
                waits = list(si.on_wait) if (si is not None and si.on_wait) else []
                if waits:
                    keep = 0 if inst.opcode == "Drain" else max_keep
                    kept, hoisted = [], []
                    for w in waits:
                        if len(kept) < keep and w.wait_mode == "sem-ge-imm":
                            kept.append(w)
                        else:
                            hoisted.append(w)
                    if hoisted:
                        for w in hoisted:
                            _hoist_ctr[0] += 1
                            ev = mybir.InstEventSemaphore(
                                name=f"I-hoistw-{_hoist_ctr[0]}", ins=[], outs=[]
                            )
                            ev.engine = inst.engine
                            ev.sync_info = mybir.SyncInfo(on_wait=[w], on_update=[])
                            out.append(ev)
                        si.on_wait = kept
                        changed = True
                out.append(inst)
            if changed:
                bb.instructions = out


# ---------------------------------------------------------------------------
# Kernel build
# ---------------------------------------------------------------------------

def build_program(debug=False):
    nc = bass.Bass(num_devices=NCORES)
    groups = [list(range(NCORES))]

    # raise Tile's stale SBUF cap (cayman has 208 KB usable per partition)
    import concourse.tile_utils as tile_utils
    if getattr(tile_utils, "max_sbuf_usage", 0) < 200 * 1024:
        tile_utils.max_sbuf_usage = 200 * 1024

    xs = nc.dram_tensor("xs", [BL, N, D], F32, kind="ExternalInput")
    ms = nc.dram_tensor("ms", [SL, D], F32, kind="ExternalInput")
    convw = nc.dram_tensor("convw", [D, D], F32, kind="ExternalInput")
    convb = nc.dram_tensor("convb", [D], F32, kind="ExternalInput")
    scal = nc.dram_tensor("scal", [1], F32, kind="ExternalInput")
    identb_in = nc.dram_tensor("identb", [128, 128], BF16, kind="ExternalInput")
    out_ext = nc.dram_tensor("out", [BL, N, D], BF16, kind="ExternalOutput")

    # collective bounce buffers
    warm_in = nc.dram_tensor("warm_in", [8, 4], F32)
    warm_out = nc.dram_tensor("warm_out", [8, 4], F32, addr_space="Shared")
    q_in = nc.dram_tensor("q_in", [128, BL], F32)
    q_ag = nc.dram_tensor("q_ag", [128 * NCORES, BL], F32, addr_space="Shared")
    cand_in = nc.dram_tensor("cand_in", [B, 64], F32)
    cand_ag = nc.dram_tensor("cand_ag", [B * NCORES, 64], F32, addr_space="Shared")
    proto_in = nc.dram_tensor("proto_in", [B, D], F32)
    proto_rs = nc.dram_tensor("proto_rs", [BL, D], F32)

    with tile.TileContext(nc) as tc, ExitStack() as top:
        # warmup collective FIRST: no input DMA (contents unused), so the
        # trigger has no dependencies and fires at t~0, absorbing the
        # collectives-subsystem cold start under Phase A.
        nc.gpsimd.collective_compute(
            "AllReduce", ALU.add, replica_groups=groups,
            ins=[warm_in[:]], outs=[warm_out[:]],
        )

        cst = top.enter_context(tc.tile_pool(name="cst", bufs=1))
        big = top.enter_context(tc.tile_pool(name="big", bufs=1))
        sml = top.enter_context(tc.tile_pool(name="sml", bufs=1))

        # constants on the scalar HWDGE ring so the sync ring starts x
        # loads immediately.
        identb = cst.tile([128, 128], BF16)
        nc.scalar.dma_start(identb[:], identb_in[:])
        ones = cst.tile([128, 128], F32)
        nc.gpsimd.memset(ones[:], 1.0)
        zeros = cst.tile([128, 1], F32)
        nc.gpsimd.memset(zeros[:], 0.0)
        bias_col = cst.tile([128, 1], F32)
        nc.scalar.dma_start(bias_col[:], convb[:].rearrange("(p o) -> p o", o=1))
        scal_sb = cst.tile([1, 1], F32)
        nc.scalar.dma_start(scal_sb[:], scal[:].rearrange("(p o) -> p o", o=1))

        # conv_w -> WT bf16 in SBUF (cast + single bf16 PE transpose)
        wconv = cst.tile([128, 128], F32)
        nc.scalar.dma_start(wconv[:], convw[:])
        wconv_b = cst.tile([128, 128], BF16)
        nc.vector.tensor_copy(wconv_b[:], wconv[:])
        wt_conv = cst.tile([128, 128], BF16)

        # persistent SBUF
        xb = [big.tile([128, N], BF16, name=f"xb{b}", tag=f"xb{b}")
              for b in range(BL)]                      # 8 KB/part each
        sim_sb = big.tile([128, 4096], BF16)           # 8 KB/part (fold-2)
        mraw = big.tile([128, SL], BF16)               # raw memory bf16, 16 KB/part
        wb_t = big.tile([128, 4096], BF16)             # masked softmax W, 8 KB/part
        mtsb = big.tile([128, SL], BF16)               # normalized memory^T, 16 KB/part
        qacc = sml.tile([128, 32], F32)
        qT_all = sml.tile([128, B], F32)
        qTb = sml.tile([128, B], BF16)
        cands = sml.tile([128, 32], F32)
        cand_all = sml.tile([B, NCORES * 64], F32)
        mr_scr = sml.tile([B, NCORES * 64], F32)
        t16 = sml.tile([B, 16], F32)
        e16 = sml.tile([B, 16], F32)
        params = sml.tile([128, 4], F32)
        ssq = sml.tile([128, MT], F32)
        minv = sml.tile([128, MT], F32)
        proto_sb = sml.tile([B, D], F32)
        proto_loc = sml.tile([1, BL * D], F32)
        cwork = sml.tile([64, 8], F32)

        scal_col = cst.tile([128, 1], F32)
        with tc.tile_pool(name="wt0ps", bufs=1, space="PSUM") as wt0ps:
            wtp = wt0ps.tile([128, 128], BF16)
            nc.tensor.transpose(wtp[:], wconv_b[:], identb[:])
            nc.vector.tensor_copy(wt_conv[:], wtp[:])
            scp = wt0ps.tile([128, 1], F32)
            nc.tensor.matmul(scp[:], ones[0:1, :], scal_sb[0:1, 0:1],
                             start=True, stop=True)
            nc.vector.tensor_copy(scal_col[:], scp[:])

        # ---- Phases A+B interleaved -------------------------------------
        with ExitStack() as pa:
            xstp = pa.enter_context(tc.tile_pool(name="xst", bufs=2))
            xt_sbp = pa.enter_context(tc.tile_pool(name="xt_sb", bufs=4))
            gelp = pa.enter_context(tc.tile_pool(name="gel", bufs=2))
            xt_ps = pa.enter_context(tc.tile_pool(name="xt_ps", bufs=4, space="PSUM"))
            ft_ps = pa.enter_context(tc.tile_pool(name="ft_ps", bufs=2, space="PSUM"))
            m_in = pa.enter_context(tc.tile_pool(name="m_in", bufs=2))
            sq_p = pa.enter_context(tc.tile_pool(name="sq", bufs=2))

            def emit_b_load(c):
                # memory chunk c (1024 rows): load + bf16 raw copy (ACT Copy,
                # in every act table set) + squared row norms (DVE TT-square +
                # grouped reduce).  Sqrt/normalize/transpose are all deferred
                # past the loop into the q-AllGather shadow, so no act-table
                # thrashing and no PE stalls here.
                mi = m_in.tile([128, 1024], F32, name="mi", tag="mi")
                nc.sync.dma_start(
                    mi[:].rearrange("p (t d) -> p t d", d=128),
                    ms[c * 1024:(c + 1) * 1024].rearrange("(t p) d -> p t d",
                                                          p=128),
                )
                nc.scalar.copy(mraw[:, c * 1024:(c + 1) * 1024], mi[:])
                sq = sq_p.tile([128, 1024], BF16, name="sq", tag="sq")
                nc.vector.tensor_tensor(sq[:], mi[:], mi[:], op=ALU.mult)
                nc.vector.tensor_reduce(
                    ssq[:, c * 8:c * 8 + 8],
                    sq[:].rearrange("p (t d) -> p t d", d=128),
                    axis=mybir.AxisListType.X, op=ALU.add,
                )

            def emit_a_batch(b):
                # batch b: load [128, 4096] f32 (p-outer: partition p owns
                # tokens p*32..p*32+31 -> contiguous 16KB DMA lines), cast to
                # bf16 (DVE, all 4 groups up front), then ALL transposes
                # before ALL feat matmuls so a matmul waiting on its PSUM
                # copy never blocks the next transpose in the PE FIFO.
                xstage = xstp.tile([128, N], F32)
                if b == 0:
                    # split the first load so compute starts ~4x earlier
                    for j in range(4):
                        nc.sync.dma_start(
                            xstage[:, j * 1024:(j + 1) * 1024].rearrange(
                                "p (t d) -> p t d", d=128),
                            xs[b].rearrange("(p t) d -> p t d", p=128)[
                                :, j * 8:(j + 1) * 8, :],
                        )
                else:
                    nc.sync.dma_start(
                        xstage[:].rearrange("p (t d) -> p t d", d=128),
                        xs[b].rearrange("(p t) d -> p t d", p=128),
                    )
                for j in range(4):
                    nc.vector.tensor_copy(
                        xb[b][:, j * 1024:(j + 1) * 1024],
                        xstage[:, j * 1024:(j + 1) * 1024],
                    )
                xsbs = []
                for j in range(4):          # 1024-col groups
                    base = j * 1024
                    xp = xt_ps.tile([128, 1024], BF16, name="xp", tag="xp")
                    for k in range(8):
                        nc.tensor.transpose(
                            xp[:, k * 128:(k + 1) * 128],
                            xb[b][:, base + k * 128:base + (k + 1) * 128],
                            identb[:],
                        )
                    xsb = xt_sbp.tile([128, 1024], BF16)
                    nc.vector.tensor_copy(xsb[:], xp[:])
                    xsbs.append(xsb)
                for j in range(4):
                    xsb = xsbs[j]
                    fp = ft_ps.tile([128, 1024], F32)
                    nc.tensor.matmul(fp[:, 0:512], wt_conv[:], xsb[:, 0:512],
                                     start=True, stop=True)
                    nc.tensor.matmul(fp[:, 512:1024], wt_conv[:],
                                     xsb[:, 512:1024], start=True, stop=True)
                    gl = gelp.tile([128, 1024], BF16, name="gl", tag="gl")
                    col = 4 * b + j
                    nc.scalar.activation(
                        gl[:], fp[:], AF.Gelu,
                        bias=bias_col[:], accum_out=qacc[:, col:col + 1],
                    )

            for b in range(BL):
                emit_a_batch(b)
                emit_b_load(b)

            qT = sml.tile([128, BL], F32)
            nc.vector.tensor_reduce(
                qT[:], qacc[:].rearrange("p (b g) -> p b g", g=4),
                axis=mybir.AxisListType.X, op=ALU.add,
            )
            nc.sync.dma_start(q_in[:], qT[:])

        nc.gpsimd.collective_compute(
            "AllGather", ALU.bypass, replica_groups=groups,
            ins=[q_in[:]], outs=[q_ag[:]],
        )

        # deferred memory normalization + transpose, in the q-AllGather
        # shadow (also keeps the PE busy so the HAM clock-gate stays open):
        # one batched sqrt (single act-table switch), then per chunk one
        # broadcast multiply from the resident bf16 mraw + 8 PE transposes.
        nc.scalar.activation(minv[:], ssq[:], AF.Sqrt, bias=zeros[:])
        nc.vector.reciprocal(minv[:], minv[:])
        with ExitStack() as pm:
            mn_p = pm.enter_context(tc.tile_pool(name="mn", bufs=2))
            mt_ps = pm.enter_context(tc.tile_pool(name="mt_ps", bufs=2,
                                                  space="PSUM"))
            for c in range(MC // 2):
                mn = mn_p.tile([128, 1024], BF16, name="mn", tag="mn")
                iv = minv[:, c * 8:c * 8 + 8]
                nc.vector.tensor_tensor(
                    mn[:].rearrange("p (t d) -> p t d", d=128),
                    mraw[:, c * 1024:(c + 1) * 1024].rearrange(
                        "p (t d) -> p t d", d=128),
                    iv.rearrange("p (t o) -> p t o", o=1).broadcast_to(
                        [128, 8, 128]),
                    op=ALU.mult,
                )
                mp = mt_ps.tile([128, 1024], BF16)
                for k in range(8):
                    nc.tensor.transpose(
                        mp[:, k * 128:(k + 1) * 128],
                        mn[:, k * 128:(k + 1) * 128], identb[:],
                    )
                nc.vector.tensor_copy(mtsb[:, c * 1024:(c + 1) * 1024], mp[:])

        nc.sync.dma_start(
            qT_all[:].rearrange("p (c b) -> p c b", c=NCORES),
            q_ag[:].rearrange("(c p) b -> p c b", p=128),
        )
        nc.vector.tensor_copy(qTb[:], qT_all[:])

        # cinv = 1/||q_b||
        qsq = sml.tile([128, B], F32)
        nc.vector.tensor_tensor(qsq[:], qT_all[:], qT_all[:], op=ALU.mult)
        with tc.tile_pool(name="nrm_ps", bufs=1, space="PSUM") as nrmp:
            nrm = nrmp.tile([1, B], F32)
            nc.tensor.matmul(nrm[:], ones[:, 0:1], qsq[:], start=True, stop=True)
            nrow = sml.tile([1, B], F32)
            nc.scalar.activation(nrow[:], nrm[:], AF.Sqrt, bias=zeros[0:1, :])
            nc.vector.reciprocal(nrow[:], nrow[:])
            ncol = nrmp.tile([B, 1], F32)
            nc.tensor.matmul(ncol[:], nrow[:], ones[0:1, 0:1],
                             start=True, stop=True)
            nc.vector.tensor_copy(params[0:B, 0:1], ncol[:])

        # ---- sim matmuls (fold-2 into 128-part psum tiles) ---------------
        with tc.tile_pool(name="sim_ps", bufs=2, space="PSUM") as sim_ps:
            for cc in range(MC // 2):
                sp = sim_ps.tile([128, 512], F32)
                for half in range(2):
                    c = half * (MC // 2) + cc
                    nc.tensor.matmul(sp[half * 64:half * 64 + 64, :],
                                     qTb[:], mtsb[:, c * 512:(c + 1) * 512],
                                     start=True, stop=True)
                nc.vector.tensor_copy(sim_sb[:, cc * 512:(cc + 1) * 512], sp[:])

        for blk in range(4):
            nc.vector.max(
                cands[:, blk * 8:(blk + 1) * 8],
                sim_sb[:, blk * 1024:(blk + 1) * 1024],
            )
        nc.sync.dma_start(cand_in[:, 0:32], cands[0:64, :])
        nc.sync.dma_start(cand_in[:, 32:64], cands[64:128, :])

        nc.gpsimd.collective_compute(
            "AllGather", ALU.bypass, replica_groups=groups,
            ins=[cand_in[:]], outs=[cand_ag[:]],
        )
        nc.sync.dma_start(
            cand_all[:].rearrange("b (c j) -> b c j", c=NCORES),
            cand_ag[:].rearrange("(c b) j -> b c j", b=B),
        )

        # ---- merge: global top-16, softmax scalars -----------------------
        nc.vector.max(t16[:, 0:8], cand_all[:])
        nc.vector.match_replace(mr_scr[:], t16[:, 0:8], cand_all[:], NEG_BIG)
        nc.vector.max(t16[:, 8:16], mr_scr[:])

        nc.vector.tensor_tensor(cwork[:, 0:1], t16[:, 0:1], params[0:B, 0:1],
                                op=ALU.mult)
        nc.vector.tensor_scalar_mul(cwork[:, 1:2], cwork[:, 0:1], -1.0)
        nc.scalar.activation(e16[:], t16[:], AF.Exp,
                             bias=cwork[:, 1:2], scale=params[0:B, 0:1])
        nc.vector.tensor_reduce(cwork[:, 2:3], e16[:],
                                axis=mybir.AxisListType.X, op=ALU.add)
        nc.scalar.activation(cwork[:, 3:4], cwork[:, 2:3], AF.Ln,
                             bias=zeros[0:B, :])
        nc.vector.tensor_tensor(params[0:B, 1:2], cwork[:, 1:2], cwork[:, 3:4],
                                op=ALU.subtract)
        nc.vector.tensor_copy(params[0:B, 2:3], t16[:, 15:16])
        nc.sync.dma_start(params[64:128, 0:3], params[0:64, 0:3])

        if debug:
            dbg_t16 = nc.dram_tensor("dbg_t16", [B, 16], F32,
                                     kind="ExternalOutput")
            dbg_params = nc.dram_tensor("dbg_params", [128, 4], F32,
                                        kind="ExternalOutput")
            dbg_proto = nc.dram_tensor("dbg_proto", [B, D], F32,
                                       kind="ExternalOutput")
            nc.sync.dma_start(dbg_t16[:], t16[:])
            nc.sync.dma_start(dbg_params[:], params[:])
            nc.sync.dma_start(dbg_proto[:], proto_sb[:])

        # ---- Phase D: dense masked softmax W -> partial proto ------------
        with ExitStack() as pd:
            maskp = pd.enter_context(tc.tile_pool(name="mask", bufs=2))
            wt_sbp = pd.enter_context(tc.tile_pool(name="wt_sb", bufs=2))
            wt_psp = pd.enter_context(tc.tile_pool(name="wt_ps", bufs=2, space="PSUM"))
            pr_ps = pd.enter_context(tc.tile_pool(name="pr_ps", bufs=1, space="PSUM"))

            for quar in range(4):
                qs = slice(quar * 1024, (quar + 1) * 1024)
                mk = maskp.tile([128, 1024], BF16)
                nc.vector.tensor_scalar(
                    mk[:], sim_sb[:, qs], params[:, 2:3], None, op0=ALU.is_ge
                )
                nc.scalar.activation(
                    wb_t[:, qs], sim_sb[:, qs], AF.Exp,
                    bias=params[:, 1:2], scale=params[:, 0:1],
                )
                nc.vector.tensor_tensor(
                    wb_t[:, qs], wb_t[:, qs], mk[:], op=ALU.mult
                )

            pr = pr_ps.tile([64, 128], F32)
            for half in range(2):
                for k0 in range(0, 32, 8):
                    idh = identb[half * 64:half * 64 + 64,
                                 half * 64:half * 64 + 64]
                    wps = wt_psp.tile([128, 512], BF16)
                    for kk in range(8):
                        k = k0 + kk
                        nc.tensor.transpose(
                            wps[:, kk * 64:(kk + 1) * 64],
                            wb_t[half * 64:half * 64 + 64,
                                 k * 128:(k + 1) * 128],
                            idh,
                        )
                    wsb = wt_sbp.tile([128, 512], BF16)
                    nc.vector.tensor_copy(wsb[:], wps[:])
                    for kk in range(8):
                        t = half * 32 + k0 + kk
                        nc.tensor.matmul(
                            pr[:], wsb[:, kk * 64:(kk + 1) * 64],
                            mraw[:, t * 128:(t + 1) * 128],
                            start=(t == 0), stop=(t == MT - 1),
                        )
            nc.vector.tensor_copy(proto_sb[:], pr[:])
            nc.sync.dma_start(proto_in[:], proto_sb[:])

        nc.gpsimd.collective_compute(
            "ReduceScatter", ALU.add, replica_groups=groups,
            ins=[proto_in[:]], outs=[proto_rs[:]],
        )
        nc.sync.dma_start(proto_loc[:], proto_rs[:].rearrange("b d -> (b d)")
                          .rearrange("(o f) -> o f", o=1))

        # ---- Phase E: out = x + scale * proto broadcast (bf16) -----------
        with tc.tile_pool(name="bb_ps", bufs=2, space="PSUM") as bbp, \
             tc.tile_pool(name="bb_sb", bufs=2) as bbs:
            for b in range(BL):
                pb_ = bbp.tile([128, 128], F32)
                nc.tensor.matmul(pb_[:], ones[0:1, :],
                                 proto_loc[0:1, b * 128:(b + 1) * 128],
                                 start=True, stop=True)
                pbs = bbs.tile([128, 128], BF16)
                nc.vector.tensor_scalar(pbs[:], pb_[:], scal_col[:, 0:1],
                                        None, op0=ALU.mult)
                seg = xb[b][:].rearrange("p (t d) -> p t d", d=128)
                nc.vector.tensor_tensor(
                    seg, seg,
                    pbs[:].rearrange("p (o d) -> p o d", o=1).broadcast_to(
                        [128, N // 128, 128]
                    ),
                    op=ALU.add,
                )
                nc.sync.dma_start(
                    out_ext[b].rearrange("(p t) d -> p t d", p=128),
                    seg,
                )

    _hoist_waits(nc)
    return nc


_CACHED = {}


def kernel(x, conv_w, conv_b, memory, retrieval_scale):
    import ml_dtypes
    x = np.ascontiguousarray(np.asarray(x, dtype=np.float32))
    conv_w = np.ascontiguousarray(np.asarray(conv_w, dtype=np.float32))
    conv_b = np.ascontiguousarray(np.asarray(conv_b, dtype=np.float32))
    memory = np.ascontiguousarray(np.asarray(memory, dtype=np.float32))
    scal = np.asarray(retrieval_scale, dtype=np.float32).reshape(1)
    identb = np.eye(128, dtype=ml_dtypes.bfloat16)

    if "nc" not in _CACHED:
        _CACHED["nc"] = build_program()
    nc = _CACHED["nc"]

    in_maps = []
    for c in range(NCORES):
        in_maps.append({
            "xs": x[c * BL:(c + 1) * BL],
            "ms": memory[c * SL:(c + 1) * SL],
            "convw": conv_w,
            "convb": conv_b,
            "scal": scal,
            "identb": identb,
        })
    res = run_bass_kernel_spmd(nc, in_maps, list(range(NCORES)),
                               **_CACHED.get("run_kwargs", {}))
    _CACHED["last_result"] = res
    out = np.empty_like(x)
    for c in range(NCORES):
        out[c * BL:(c + 1) * BL] = np.asarray(res.results[c]["out"],
                                              dtype=np.float32)
    return out


# revision 19
# speedup vs baseline: 1.1485x; 1.0319x over previous
"""Trainium2 Bass kernel for nn_BPBookMemory (retrieval_knn).

Strategy (8 NeuronCores, SPMD):
  - x sharded by batch (8 per core); memory bank sharded 8-way (8192 rows/core).
  - Warmup collective triggered at t=0 (no input DMA) so the ~60us cold-start
    of the collectives subsystem overlaps Phase A instead of serializing.
  - Phase A: stream x (p-outer layout: each partition owns a contiguous
    32-token block -> 16KB DMA lines), cast to bf16 on GpSimd, PE-transpose,
    featT = gelu(W xT + b), accumulate q sums per batch on ACT (accum_out).
  - Phase B (interleaved with A in emission order so it overlaps): load
    memory shard, bf16 raw copy (GpSimd), row norms (ACT square+accum),
    normalize (DVE), PE-transpose -> mt tiles.
  - AllGather q -> all 64 query vectors everywhere.
  - sim[b, s_local] matmuls for all 64 batches; block-wise max8 gives 64
    candidate values per batch per core.
  - AllGather candidates -> identical merge on every core via max8 +
    match_replace + max8 -> global top-16 values, threshold, softmax scalars.
  - Dense masked softmax weights W = mask * exp(...) in bf16, PE-transpose,
    partial proto = W @ memory_shard; ReduceScatter(add).
  - out = x + retrieval_scale * proto, stored as bf16 (upcast to f32 on host;
    bf16 rounding of the output is ~0.2% rel, far under the 2e-2 gate).

Index-free top-k: only candidate VALUES travel; selection is by threshold
(sim >= 16th-largest), so no max_index / gather is ever needed.
"""

import os
import sys

for _p in ("/opt/trn_rl_repo", "/root/.axon_site/_ro/trn_rl_repo"):
    if os.path.isdir(_p) and _p not in sys.path:
        sys.path.append(_p)

import numpy as np
from contextlib import ExitStack

import concourse.bass as bass
import concourse.tile as tile
from concourse import mybir
from concourse.bass_utils import run_bass_kernel_spmd
from concourse.vector_clock import ScopedClock

F32 = mybir.dt.float32
BF16 = mybir.dt.bfloat16
AF = mybir.ActivationFunctionType
ALU = mybir.AluOpType

NCORES = 8
B, N, D, S = 64, 4096, 128, 65536
BL = B // NCORES          # 8 batches per core
SL = S // NCORES          # 8192 memory rows per core
MT = SL // 128            # 64 memory tiles per core
MC = SL // 512            # 16 memory chunks of 512
NEG_BIG = -1.0e30


# ---------------------------------------------------------------------------
# Walrus workaround: this container's neuronxcc rejects instructions carrying
# more than ~1 sync wait command (Drain/TPB_CTRL, LDWEIGHTS/S3_LW...).
# 1) Replace Tile's exit drain+barrier with EventSemaphore-carried waits.
# 2) Post-pass: hoist excess waits onto standalone EventSemaphore insts.
# ---------------------------------------------------------------------------

def _patched_drain_and_barrier(self, tick_clock, wait_clock):
    nc = self.nc
    carrier = nc.sync.add_instruction(
        mybir.InstEventSemaphore(name=f"I-{nc.next_id()}", ins=[], outs=[])
    )
    wait_clock.add_sem_waits(carrier.ins, ScopedClock({None: tick_clock.global_clock}))
    si = carrier.ins.sync_info
    waits = list(si.on_wait or [])
    if len(waits) > 1:
        si.on_wait = [waits[0]]
        for w in waits[1:]:
            extra = nc.sync.add_instruction(
                mybir.InstEventSemaphore(name=f"I-{nc.next_id()}", ins=[], outs=[])
            )
            extra.ins.sync_info = mybir.SyncInfo(on_wait=[w], on_update=[])
    for eng in nc.engines.values():
        eng.drain()
    nc.all_engine_barrier(sem_only=True)
    popped = nc._tile_sem_poison_stack.pop()
    assert popped is self._sem_poison
    nc.clear_and_free_semaphores(list(self.sems.allocated().values()))
    nc.all_engine_barrier(sem_only=True)


tile.TileContext._drain_and_barrier = _patched_drain_and_barrier

_hoist_ctr = [0]


def _hoist_waits(nc, max_keep=1):
    for f in nc.m.functions:
        for bb in f.blocks:
            insts = bb.instructions
            out = []
            changed = False
            for inst in insts:
                si = inst.sync_info
                waits = list(si.on_wait) if (si is not None and si.on_wait) else []
                if waits:
                    keep = 0 if inst.opcode == "Drain" else max_keep
                    kept, hoisted = [], []
                    for w in waits:
                        if len(kept) < keep and w.wait_mode == "sem-ge-imm":
                            kept.append(w)
                        else:
                            hoisted.append(w)
                    if hoisted:
                        for w in hoisted:
                            _hoist_ctr[0] += 1
                            ev = mybir.InstEventSemaphore(
                                name=f"I-hoistw-{_hoist_ctr[0]}", ins=[], outs=[]
                            )
                            ev.engine = inst.engine
                            ev.sync_info = mybir.SyncInfo(on_wait=[w], on_update=[])
                            out.append(ev)
                        si.on_wait = kept
                        changed = True
                out.append(inst)
            if changed:
                bb.instructions = out


# ---------------------------------------------------------------------------
# Kernel build
# ---------------------------------------------------------------------------

def build_program(debug=False):
    nc = bass.Bass(num_devices=NCORES)
    groups = [list(range(NCORES))]

    # raise Tile's stale SBUF cap (cayman has 208 KB usable per partition)
    import concourse.tile_utils as tile_utils
    if getattr(tile_utils, "max_sbuf_usage", 0) < 200 * 1024:
        tile_utils.max_sbuf_usage = 200 * 1024

    xs = nc.dram_tensor("xs", [BL, N, D], F32, kind="ExternalInput")
    ms = nc.dram_tensor("ms", [SL, D], F32, kind="ExternalInput")
    convw = nc.dram_tensor("convw", [D, D], F32, kind="ExternalInput")
    convb = nc.dram_tensor("convb", [D], F32, kind="ExternalInput")
    scal = nc.dram_tensor("scal", [1], F32, kind="ExternalInput")
    identb_in = nc.dram_tensor("identb", [128, 128], BF16, kind="ExternalInput")
    out_ext = nc.dram_tensor("out", [BL, N, D], BF16, kind="ExternalOutput")

    # collective bounce buffers
    warm_in = nc.dram_tensor("warm_in", [8, 4], F32)
    warm_out = nc.dram_tensor("warm_out", [8, 4], F32, addr_space="Shared")
    q_in = nc.dram_tensor("q_in", [128, BL], F32)
    q_ag = nc.dram_tensor("q_ag", [128 * NCORES, BL], F32, addr_space="Shared")
    cand_in = nc.dram_tensor("cand_in", [B, 64], F32)
    cand_ag = nc.dram_tensor("cand_ag", [B * NCORES, 64], F32, addr_space="Shared")
    proto_in = nc.dram_tensor("proto_in", [B, D], F32)
    proto_rs = nc.dram_tensor("proto_rs", [BL, D], F32)

    with tile.TileContext(nc) as tc, ExitStack() as top:
        # warmup collective FIRST: no input DMA (contents unused), so the
        # trigger has no dependencies and fires at t~0, absorbing the
        # collectives-subsystem cold start under Phase A.
        nc.gpsimd.collective_compute(
            "AllReduce", ALU.add, replica_groups=groups,
            ins=[warm_in[:]], outs=[warm_out[:]],
        )

        cst = top.enter_context(tc.tile_pool(name="cst", bufs=1))
        big = top.enter_context(tc.tile_pool(name="big", bufs=1))
        sml = top.enter_context(tc.tile_pool(name="sml", bufs=1))

        # constants on the scalar HWDGE ring so the sync ring starts x
        # loads immediately.  conv_w first: the whole feat pipeline waits
        # on its transposed copy.
        wconv = cst.tile([128, 128], F32)
        nc.scalar.dma_start(wconv[:], convw[:])
        identb = cst.tile([128, 128], BF16)
        nc.scalar.dma_start(identb[:], identb_in[:])
        ones = cst.tile([128, 128], F32)
        nc.gpsimd.memset(ones[:], 1.0)
        zeros = cst.tile([128, 1], F32)
        nc.gpsimd.memset(zeros[:], 0.0)
        bias_col = cst.tile([128, 1], F32)
        nc.scalar.dma_start(bias_col[:], convb[:].rearrange("(p o) -> p o", o=1))
        scal_sb = cst.tile([1, 1], F32)
        nc.scalar.dma_start(scal_sb[:], scal[:].rearrange("(p o) -> p o", o=1))

        # conv_w -> WT bf16 in SBUF (cast + single bf16 PE transpose);
        # the small setup copies run on GpSimd, which is otherwise idle,
        # so they never get stuck behind Phase A work in the DVE FIFO.
        wconv_b = cst.tile([128, 128], BF16)
        nc.gpsimd.tensor_copy(wconv_b[:], wconv[:])
        wt_conv = cst.tile([128, 128], BF16)

        # persistent SBUF
        xb = [big.tile([128, N], BF16, name=f"xb{b}", tag=f"xb{b}")
              for b in range(BL)]                      # 8 KB/part each
        sim_sb = big.tile([128, 4096], BF16)           # 8 KB/part (fold-2)
        mraw = big.tile([128, SL], BF16)               # raw memory bf16, 16 KB/part
        wb_t = big.tile([128, 4096], BF16)             # masked softmax W, 8 KB/part
        mtsb = big.tile([128, SL], BF16)               # normalized memory^T, 16 KB/part
        qacc = sml.tile([128, 32], F32)
        qT_all = sml.tile([128, B], F32)
        qTb = sml.tile([128, B], BF16)
        cands = sml.tile([128, 32], F32)
        cand_all = sml.tile([B, NCORES * 64], F32)
        mr_scr = sml.tile([B, NCORES * 64], F32)
        t16 = sml.tile([B, 16], F32)
        e16 = sml.tile([B, 16], F32)
        params = sml.tile([128, 4], F32)
        ssq = sml.tile([128, MT], F32)
        minv = sml.tile([128, MT], F32)
        proto_sb = sml.tile([B, D], F32)
        proto_loc = sml.tile([1, BL * D], F32)
        cwork = sml.tile([64, 8], F32)

        scal_col = cst.tile([128, 1], F32)
        with tc.tile_pool(name="wt0ps", bufs=1, space="PSUM") as wt0ps:
            wtp = wt0ps.tile([128, 128], BF16)
            nc.tensor.transpose(wtp[:], wconv_b[:], identb[:])
            nc.scalar.copy(wt_conv[:], wtp[:])
            scp = wt0ps.tile([128, 1], F32)
            nc.tensor.matmul(scp[:], ones[0:1, :], scal_sb[0:1, 0:1],
                             start=True, stop=True)
            nc.scalar.copy(scal_col[:], scp[:])

        # ---- Phases A+B interleaved -------------------------------------
        with ExitStack() as pa:
            xstp = pa.enter_context(tc.tile_pool(name="xst", bufs=2))
            xt_sbp = pa.enter_context(tc.tile_pool(name="xt_sb", bufs=4))
            gelp = pa.enter_context(tc.tile_pool(name="gel", bufs=2))
            xt_ps = pa.enter_context(tc.tile_pool(name="xt_ps", bufs=4, space="PSUM"))
            ft_ps = pa.enter_context(tc.tile_pool(name="ft_ps", bufs=2, space="PSUM"))
            m_in = pa.enter_context(tc.tile_pool(name="m_in", bufs=2))
            sq_p = pa.enter_context(tc.tile_pool(name="sq", bufs=2))

            def emit_b_load(c):
                # memory chunk c (1024 rows): load + bf16 raw copy (ACT Copy,
                # in every act table set) + squared row norms (DVE TT-square +
                # grouped reduce).  Sqrt/normalize/transpose are all deferred
                # past the loop into the q-AllGather shadow, so no act-table
                # thrashing and no PE stalls here.
                mi = m_in.tile([128, 1024], F32, name="mi", tag="mi")
                nc.sync.dma_start(
                    mi[:].rearrange("p (t d) -> p t d", d=128),
                    ms[c * 1024:(c + 1) * 1024].rearrange("(t p) d -> p t d",
                                                          p=128),
                )
                nc.scalar.copy(mraw[:, c * 1024:(c + 1) * 1024], mi[:])
                sq = sq_p.tile([128, 1024], BF16, name="sq", tag="sq")
                nc.vector.tensor_tensor(sq[:], mi[:], mi[:], op=ALU.mult)
                nc.vector.tensor_reduce(
                    ssq[:, c * 8:c * 8 + 8],
                    sq[:].rearrange("p (t d) -> p t d", d=128),
                    axis=mybir.AxisListType.X, op=ALU.add,
                )

            def emit_a_batch(b):
                # batch b: load [128, 4096] f32 (p-outer: partition p owns
                # tokens p*32..p*32+31 -> contiguous 16KB DMA lines), cast to
                # bf16 (DVE, all 4 groups up front), then ALL transposes
                # before ALL feat matmuls so a matmul waiting on its PSUM
                # copy never blocks the next transpose in the PE FIFO.
                xstage = xstp.tile([128, N], F32)
                if b == 0:
                    # split the first load so compute starts ~4x earlier
                    for j in range(4):
                        nc.sync.dma_start(
                            xstage[:, j * 1024:(j + 1) * 1024].rearrange(
                                "p (t d) -> p t d", d=128),
                            xs[b].rearrange("(p t) d -> p t d", p=128)[
                                :, j * 8:(j + 1) * 8, :],
                        )
                else:
                    nc.sync.dma_start(
                        xstage[:].rearrange("p (t d) -> p t d", d=128),
                        xs[b].rearrange("(p t) d -> p t d", p=128),
                    )
                for j in range(4):
                    nc.vector.tensor_copy(
                        xb[b][:, j * 1024:(j + 1) * 1024],
                        xstage[:, j * 1024:(j + 1) * 1024],
                    )
                xsbs = []
                for j in range(4):          # 1024-col groups
                    base = j * 1024
                    xp = xt_ps.tile([128, 1024], BF16, name="xp", tag="xp")
                    for k in range(8):
                        nc.tensor.transpose(
                            xp[:, k * 128:(k + 1) * 128],
                            xb[b][:, base + k * 128:base + (k + 1) * 128],
                            identb[:],
                        )
                    xsb = xt_sbp.tile([128, 1024], BF16)
                    nc.vector.tensor_copy(xsb[:], xp[:])
                    xsbs.append(xsb)
                for j in range(4):
                    xsb = xsbs[j]
                    fp = ft_ps.tile([128, 1024], F32)
                    nc.tensor.matmul(fp[:, 0:512], wt_conv[:], xsb[:, 0:512],
                                     start=True, stop=True)
                    nc.tensor.matmul(fp[:, 512:1024], wt_conv[:],
                                     xsb[:, 512:1024], start=True, stop=True)
                    gl = gelp.tile([128, 1024], BF16, name="gl", tag="gl")
                    col = 4 * b + j
                    nc.scalar.activation(
                        gl[:], fp[:], AF.Gelu,
                        bias=bias_col[:], accum_out=qacc[:, col:col + 1],
                    )

            for b in range(BL):
                emit_a_batch(b)
                emit_b_load(b)

            qT = sml.tile([128, BL], F32)
            nc.vector.tensor_reduce(
                qT[:], qacc[:].rearrange("p (b g) -> p b g", g=4),
                axis=mybir.AxisListType.X, op=ALU.add,
            )
            nc.sync.dma_start(q_in[:], qT[:])

        nc.gpsimd.collective_compute(
            "AllGather", ALU.bypass, replica_groups=groups,
            ins=[q_in[:]], outs=[q_ag[:]],
        )

        # deferred memory normalization + transpose, in the q-AllGather
        # shadow (also keeps the PE busy so the HAM clock-gate stays open):
        # one batched sqrt (single act-table switch), then per chunk one
        # broadcast multiply from the resident bf16 mraw + 8 PE transposes.
        nc.scalar.activation(minv[:], ssq[:], AF.Sqrt, bias=zeros[:])
        nc.vector.reciprocal(minv[:], minv[:])
        with ExitStack() as pm:
            mn_p = pm.enter_context(tc.tile_pool(name="mn", bufs=2))
            mt_ps = pm.enter_context(tc.tile_pool(name="mt_ps", bufs=2,
                                                  space="PSUM"))
            for c in range(MC // 2):
                mn = mn_p.tile([128, 1024], BF16, name="mn", tag="mn")
                iv = minv[:, c * 8:c * 8 + 8]
                nc.vector.tensor_tensor(
                    mn[:].rearrange("p (t d) -> p t d", d=128),
                    mraw[:, c * 1024:(c + 1) * 1024].rearrange(
                        "p (t d) -> p t d", d=128),
                    iv.rearrange("p (t o) -> p t o", o=1).broadcast_to(
                        [128, 8, 128]),
                    op=ALU.mult,
                )
                mp = mt_ps.tile([128, 1024], BF16)
                for k in range(8):
                    nc.tensor.transpose(
                        mp[:, k * 128:(k + 1) * 128],
                        mn[:, k * 128:(k + 1) * 128], identb[:],
                    )
                nc.vector.tensor_copy(mtsb[:, c * 1024:(c + 1) * 1024], mp[:])

        nc.sync.dma_start(
            qT_all[:].rearrange("p (c b) -> p c b", c=NCORES),
            q_ag[:].rearrange("(c p) b -> p c b", p=128),
        )
        nc.vector.tensor_copy(qTb[:], qT_all[:])

        # cinv = 1/||q_b||, broadcast to both fold rows (b and 64+b)
        qsq = sml.tile([128, B], F32)
        nc.vector.tensor_tensor(qsq[:], qT_all[:], qT_all[:], op=ALU.mult)
        with tc.tile_pool(name="nrm_ps", bufs=1, space="PSUM") as nrmp:
            nrm = nrmp.tile([1, B], F32)
            nc.tensor.matmul(nrm[:], ones[:, 0:1], qsq[:], start=True, stop=True)
            nrow2 = sml.tile([1, 2 * B], F32)
            nc.scalar.activation(nrow2[0:1, 0:B], nrm[:], AF.Sqrt,
                                 bias=zeros[0:1, :])
            nc.vector.reciprocal(nrow2[0:1, 0:B], nrow2[0:1, 0:B])
            nc.vector.tensor_copy(nrow2[0:1, B:2 * B], nrow2[0:1, 0:B])
            ncol = nrmp.tile([128, 1], F32)
            nc.tensor.matmul(ncol[:], nrow2[:], ones[0:1, 0:1],
                             start=True, stop=True)
            nc.vector.tensor_copy(params[:, 0:1], ncol[:])

        # ---- sim matmuls (fold-2 into 128-part psum tiles) ---------------
        with tc.tile_pool(name="sim_ps", bufs=2, space="PSUM") as sim_ps:
            for cc in range(MC // 2):
                sp = sim_ps.tile([128, 512], F32)
                for half in range(2):
                    c = half * (MC // 2) + cc
                    nc.tensor.matmul(sp[half * 64:half * 64 + 64, :],
                                     qTb[:], mtsb[:, c * 512:(c + 1) * 512],
                                     start=True, stop=True)
                nc.vector.tensor_copy(sim_sb[:, cc * 512:(cc + 1) * 512], sp[:])

        for blk in range(4):
            nc.vector.max(
                cands[:, blk * 8:(blk + 1) * 8],
                sim_sb[:, blk * 1024:(blk + 1) * 1024],
            )
        nc.sync.dma_start(cand_in[:, 0:32], cands[0:64, :])
        nc.sync.dma_start(cand_in[:, 32:64], cands[64:128, :])

        nc.gpsimd.collective_compute(
            "AllGather", ALU.bypass, replica_groups=groups,
            ins=[cand_in[:]], outs=[cand_ag[:]],
        )

        # dense exp in the candidate-AllGather shadow, stabilized by the
        # per-fold local max; the global correction (a per-batch rescale)
        # is folded into the threshold mask after the merge.
        lmax = sml.tile([128, 1], F32)
        lbias = sml.tile([128, 1], F32)
        rsc = sml.tile([128, 1], F32)
        nc.vector.tensor_reduce(lmax[:], cands[:],
                                axis=mybir.AxisListType.X, op=ALU.max)
        nc.vector.tensor_tensor(lbias[:], lmax[:], params[:, 0:1], op=ALU.mult)
        nc.vector.tensor_scalar_mul(lbias[:], lbias[:], -1.0)
        for quar in range(4):
            qs = slice(quar * 1024, (quar + 1) * 1024)
            nc.scalar.activation(
                wb_t[:, qs], sim_sb[:, qs], AF.Exp,
                bias=lbias[:, 0:1], scale=params[:, 0:1],
            )
        nc.sync.dma_start(
            cand_all[:].rearrange("b (c j) -> b c j", c=NCORES),
            cand_ag[:].rearrange("(c b) j -> b c j", b=B),
        )

        # ---- merge: global top-16, softmax scalars -----------------------
        nc.vector.max(t16[:, 0:8], cand_all[:])
        nc.vector.match_replace(mr_scr[:], t16[:, 0:8], cand_all[:], NEG_BIG)
        nc.vector.max(t16[:, 8:16], mr_scr[:])

        nc.vector.tensor_tensor(cwork[:, 0:1], t16[:, 0:1], params[0:B, 0:1],
                                op=ALU.mult)
        nc.vector.tensor_scalar_mul(cwork[:, 1:2], cwork[:, 0:1], -1.0)
        nc.scalar.activation(e16[:], t16[:], AF.Exp,
                             bias=cwork[:, 1:2], scale=params[0:B, 0:1])
        nc.vector.tensor_reduce(cwork[:, 2:3], e16[:],
                                axis=mybir.AxisListType.X, op=ALU.add)
        nc.scalar.activation(cwork[:, 3:4], cwork[:, 2:3], AF.Ln,
                             bias=zeros[0:B, :])
        nc.vector.tensor_tensor(params[0:B, 1:2], cwork[:, 1:2], cwork[:, 3:4],
                                op=ALU.subtract)
        nc.vector.tensor_copy(params[0:B, 2:3], t16[:, 15:16])
        nc.sync.dma_start(params[64:128, 1:3], params[0:64, 1:3])
        # rescale = exp(params1 - lbias): converts locally-stabilized exps
        # into globally-normalized softmax weights.
        nc.vector.tensor_tensor(rsc[:], params[:, 1:2], lbias[:], op=ALU.subtract)
        nc.scalar.activation(rsc[:], rsc[:], AF.Exp, bias=zeros[:])

        if debug:
            dbg_t16 = nc.dram_tensor("dbg_t16", [B, 16], F32,
                                     kind="ExternalOutput")
            dbg_params = nc.dram_tensor("dbg_params", [128, 4], F32,
                                        kind="ExternalOutput")
            dbg_proto = nc.dram_tensor("dbg_proto", [B, D], F32,
                                       kind="ExternalOutput")
            nc.sync.dma_start(dbg_t16[:], t16[:])
            nc.sync.dma_start(dbg_params[:], params[:])
            nc.sync.dma_start(dbg_proto[:], proto_sb[:])

        # ---- Phase D: dense masked softmax W -> partial proto ------------
        with ExitStack() as pd:
            maskp = pd.enter_context(tc.tile_pool(name="mask", bufs=2))
            wt_sbp = pd.enter_context(tc.tile_pool(name="wt_sb", bufs=2))
            wt_psp = pd.enter_context(tc.tile_pool(name="wt_ps", bufs=2, space="PSUM"))
            pr_ps = pd.enter_context(tc.tile_pool(name="pr_ps", bufs=1, space="PSUM"))

            for quar in range(4):
                qs = slice(quar * 1024, (quar + 1) * 1024)
                mk = maskp.tile([128, 1024], BF16)
                nc.vector.tensor_scalar(
                    mk[:], sim_sb[:, qs], params[:, 2:3], rsc[:, 0:1],
                    op0=ALU.is_ge, op1=ALU.mult,
                )
                nc.vector.tensor_tensor(
                    wb_t[:, qs], wb_t[:, qs], mk[:], op=ALU.mult
                )

            pr = pr_ps.tile([64, 128], F32)
            for half in range(2):
                for k0 in range(0, 32, 8):
                    idh = identb[half * 64:half * 64 + 64,
                                 half * 64:half * 64 + 64]
                    wps = wt_psp.tile([128, 512], BF16)
                    for kk in range(8):
                        k = k0 + kk
                        nc.tensor.transpose(
                            wps[:, kk * 64:(kk + 1) * 64],
                            wb_t[half * 64:half * 64 + 64,
                                 k * 128:(k + 1) * 128],
                            idh,
                        )
                    wsb = wt_sbp.tile([128, 512], BF16)
                    nc.vector.tensor_copy(wsb[:], wps[:])
                    for kk in range(8):
                        t = half * 32 + k0 + kk
                        nc.tensor.matmul(
                            pr[:], wsb[:, kk * 64:(kk + 1) * 64],
                            mraw[:, t * 128:(t + 1) * 128],
                            start=(t == 0), stop=(t == MT - 1),
                        )
            nc.vector.tensor_copy(proto_sb[:], pr[:])
            nc.sync.dma_start(proto_in[:], proto_sb[:])

        nc.gpsimd.collective_compute(
            "ReduceScatter", ALU.add, replica_groups=groups,
            ins=[proto_in[:]], outs=[proto_rs[:]],
        )
        nc.sync.dma_start(proto_loc[:], proto_rs[:].rearrange("b d -> (b d)")
                          .rearrange("(o f) -> o f", o=1))

        # ---- Phase E: out = x + scale * proto broadcast (bf16) -----------
        with tc.tile_pool(name="bb_ps", bufs=2, space="PSUM") as bbp, \
             tc.tile_pool(name="bb_sb", bufs=2) as bbs:
            for b in range(BL):
                pb_ = bbp.tile([128, 128], F32)
                nc.tensor.matmul(pb_[:], ones[0:1, :],
                                 proto_loc[0:1, b * 128:(b + 1) * 128],
                                 start=True, stop=True)
                pbs = bbs.tile([128, 128], BF16)
                nc.vector.tensor_scalar(pbs[:], pb_[:], scal_col[:, 0:1],
                                        None, op0=ALU.mult)
                seg = xb[b][:].rearrange("p (t d) -> p t d", d=128)
                nc.vector.tensor_tensor(
                    seg, seg,
                    pbs[:].rearrange("p (o d) -> p o d", o=1).broadcast_to(
                        [128, N // 128, 128]
                    ),
                    op=ALU.add,
                )
                nc.sync.dma_start(
                    out_ext[b].rearrange("(p t) d -> p t d", p=128),
                    seg,
                )

    _hoist_waits(nc)
    return nc


_CACHED = {}


def kernel(x, conv_w, conv_b, memory, retrieval_scale):
    import ml_dtypes
    x = np.ascontiguousarray(np.asarray(x, dtype=np.float32))
    conv_w = np.ascontiguousarray(np.asarray(conv_w, dtype=np.float32))
    conv_b = np.ascontiguousarray(np.asarray(conv_b, dtype=np.float32))
    memory = np.ascontiguousarray(np.asarray(memory, dtype=np.float32))
    scal = np.asarray(retrieval_scale, dtype=np.float32).reshape(1)
    identb = np.eye(128, dtype=ml_dtypes.bfloat16)

    if "nc" not in _CACHED:
        _CACHED["nc"] = build_program()
    nc = _CACHED["nc"]

    in_maps = []
    for c in range(NCORES):
        in_maps.append({
            "xs": x[c * BL:(c + 1) * BL],
            "ms": memory[c * SL:(c + 1) * SL],
            "convw": conv_w,
            "convb": conv_b,
            "scal": scal,
            "identb": identb,
        })
    res = run_bass_kernel_spmd(nc, in_maps, list(range(NCORES)),
                               **_CACHED.get("run_kwargs", {}))
    _CACHED["last_result"] = res
    out = np.empty_like(x)
    for c in range(NCORES):
        out[c * BL:(c + 1) * BL] = np.asarray(res.results[c]["out"],
                                              dtype=np.float32)
    return out


# revision 28
# speedup vs baseline: 1.2023x; 1.0468x over previous
"""Trainium2 Bass kernel for nn_BPBookMemory (retrieval_knn).

Strategy (8 NeuronCores, SPMD):
  - x sharded by batch (8 per core); memory bank sharded 8-way (8192 rows/core).
  - Warmup collective triggered at t=0 (no input DMA) so the ~60us cold-start
    of the collectives subsystem overlaps Phase A instead of serializing.
  - Phase A: stream x (p-outer layout: each partition owns a contiguous
    32-token block -> 16KB DMA lines), cast to bf16 on GpSimd, PE-transpose,
    featT = gelu(W xT + b), accumulate q sums per batch on ACT (accum_out).
  - Phase B (interleaved with A in emission order so it overlaps): load
    memory shard, bf16 raw copy (GpSimd), row norms (ACT square+accum),
    normalize (DVE), PE-transpose -> mt tiles.
  - AllGather q -> all 64 query vectors everywhere.
  - sim[b, s_local] matmuls for all 64 batches; block-wise max8 gives 64
    candidate values per batch per core.
  - AllGather candidates -> identical merge on every core via max8 +
    match_replace + max8 -> global top-16 values, threshold, softmax scalars.
  - Dense masked softmax weights W = mask * exp(...) in bf16, PE-transpose,
    partial proto = W @ memory_shard; ReduceScatter(add).
  - out = x + retrieval_scale * proto, stored as bf16 (upcast to f32 on host;
    bf16 rounding of the output is ~0.2% rel, far under the 2e-2 gate).

Index-free top-k: only candidate VALUES travel; selection is by threshold
(sim >= 16th-largest), so no max_index / gather is ever needed.
"""

import os
import sys

for _p in ("/opt/trn_rl_repo", "/root/.axon_site/_ro/trn_rl_repo"):
    if os.path.isdir(_p) and _p not in sys.path:
        sys.path.append(_p)

import numpy as np
from contextlib import ExitStack

import concourse.bass as bass
import concourse.tile as tile
from concourse import mybir
from concourse.bass_utils import run_bass_kernel_spmd
from concourse.vector_clock import ScopedClock

F32 = mybir.dt.float32
BF16 = mybir.dt.bfloat16
AF = mybir.ActivationFunctionType
ALU = mybir.AluOpType

NCORES = 8
B, N, D, S = 64, 4096, 128, 65536
BL = B // NCORES          # 8 batches per core
SL = S // NCORES          # 8192 memory rows per core
MT = SL // 128            # 64 memory tiles per core
MC = SL // 512            # 16 memory chunks of 512
NEG_BIG = -1.0e30


# ---------------------------------------------------------------------------
# Walrus workaround: this container's neuronxcc rejects instructions carrying
# more than ~1 sync wait command (Drain/TPB_CTRL, LDWEIGHTS/S3_LW...).
# 1) Replace Tile's exit drain+barrier with EventSemaphore-carried waits.
# 2) Post-pass: hoist excess waits onto standalone EventSemaphore insts.
# ---------------------------------------------------------------------------

def _patched_drain_and_barrier(self, tick_clock, wait_clock):
    nc = self.nc
    carrier = nc.sync.add_instruction(
        mybir.InstEventSemaphore(name=f"I-{nc.next_id()}", ins=[], outs=[])
    )
    wait_clock.add_sem_waits(carrier.ins, ScopedClock({None: tick_clock.global_clock}))
    si = carrier.ins.sync_info
    waits = list(si.on_wait or [])
    if len(waits) > 1:
        si.on_wait = [waits[0]]
        for w in waits[1:]:
            extra = nc.sync.add_instruction(
                mybir.InstEventSemaphore(name=f"I-{nc.next_id()}", ins=[], outs=[])
            )
            extra.ins.sync_info = mybir.SyncInfo(on_wait=[w], on_update=[])
    for eng in nc.engines.values():
        eng.drain()
    nc.all_engine_barrier(sem_only=True)
    popped = nc._tile_sem_poison_stack.pop()
    assert popped is self._sem_poison
    nc.clear_and_free_semaphores(list(self.sems.allocated().values()))
    nc.all_engine_barrier(sem_only=True)


tile.TileContext._drain_and_barrier = _patched_drain_and_barrier

_hoist_ctr = [0]

import bass_rust as _bass_rust
_InstISA = _bass_rust.InstISA


def _hoist_waits(nc, max_keep=1):
    for f in nc.m.functions:
        for bb in f.blocks:
            insts = bb.instructions
            out = []
            changed = False
            for inst in insts:
                si = inst.sync_info
                waits = list(si.on_wait) if (si is not None and si.on_wait) else []
                if waits:
                    # Drain and raw-ISA instructions (e.g. TensorTensorReduce)
                    # cannot carry sem waits through this walrus build.
                    keep = (0 if (inst.opcode == "Drain"
                                  or isinstance(inst, _InstISA))
                            else max_keep)
                    kept, hoisted = [], []
                    for w in waits:
                        if len(kept) < keep and w.wait_mode == "sem-ge-imm":
                            kept.append(w)
                        else:
                            hoisted.append(w)
                    if hoisted:
                        for w in hoisted:
                            _hoist_ctr[0] += 1
                            ev = mybir.InstEventSemaphore(
                                name=f"I-hoistw-{_hoist_ctr[0]}", ins=[], outs=[]
                            )
                            ev.engine = inst.engine
                            ev.sync_info = mybir.SyncInfo(on_wait=[w], on_update=[])
                            out.append(ev)
                        si.on_wait = kept
                        changed = True
                out.append(inst)
            if changed:
                bb.instructions = out


# ---------------------------------------------------------------------------
# Kernel build
# ---------------------------------------------------------------------------

def build_program(debug=False):
    nc = bass.Bass(num_devices=NCORES)
    groups = [list(range(NCORES))]

    # raise Tile's stale SBUF cap (cayman has 208 KB usable per partition)
    import concourse.tile_utils as tile_utils
    if getattr(tile_utils, "max_sbuf_usage", 0) < 200 * 1024:
        tile_utils.max_sbuf_usage = 200 * 1024

    xs = nc.dram_tensor("xs", [BL, N, D], F32, kind="ExternalInput")
    ms = nc.dram_tensor("ms", [SL, D], F32, kind="ExternalInput")
    convwt = nc.dram_tensor("convwt", [D, D], BF16, kind="ExternalInput")
    convb = nc.dram_tensor("convb", [D], F32, kind="ExternalInput")
    scalc = nc.dram_tensor("scalc", [128, 1], F32, kind="ExternalInput")
    identb_in = nc.dram_tensor("identb", [128, 128], BF16, kind="ExternalInput")
    out_ext = nc.dram_tensor("out", [BL, N, D], BF16, kind="ExternalOutput")

    # collective bounce buffers
    warm_in = nc.dram_tensor("warm_in", [8, 4], F32)
    warm_out = nc.dram_tensor("warm_out", [8, 4], F32, addr_space="Shared")
    q_in = nc.dram_tensor("q_in", [128, BL], F32)
    q_ag = nc.dram_tensor("q_ag", [128 * NCORES, BL], F32, addr_space="Shared")
    cand_in = nc.dram_tensor("cand_in", [B, 64], F32)
    cand_ag = nc.dram_tensor("cand_ag", [B * NCORES, 64], F32, addr_space="Shared")
    proto_in = nc.dram_tensor("proto_in", [B, D], F32)
    proto_rs = nc.dram_tensor("proto_rs", [BL, D], F32)

    with tile.TileContext(nc) as tc, ExitStack() as top:
        # warmup collective FIRST: no input DMA (contents unused), so the
        # trigger has no dependencies and fires at t~0, absorbing the
        # collectives-subsystem cold start under Phase A.
        nc.gpsimd.collective_compute(
            "AllReduce", ALU.add, replica_groups=groups,
            ins=[warm_in[:]], outs=[warm_out[:]],
        )

        cst = top.enter_context(tc.tile_pool(name="cst", bufs=1))
        big = top.enter_context(tc.tile_pool(name="big", bufs=1))
        sml = top.enter_context(tc.tile_pool(name="sml", bufs=1))

        # constants on the scalar HWDGE ring so the sync ring starts x
        # loads immediately.  conv_w arrives pre-transposed in bf16 and the
        # scale pre-broadcast (host-side prep), so nothing downstream waits
        # on a setup compute chain.
        wt_conv = cst.tile([128, 128], BF16)
        nc.scalar.dma_start(wt_conv[:], convwt[:])
        identb = cst.tile([128, 128], BF16)
        nc.scalar.dma_start(identb[:], identb_in[:])
        ones = cst.tile([128, 128], F32)
        nc.gpsimd.memset(ones[:], 1.0)
        zeros = cst.tile([128, 1], F32)
        nc.gpsimd.memset(zeros[:], 0.0)
        bias_col = cst.tile([128, 1], F32)
        nc.scalar.dma_start(bias_col[:], convb[:].rearrange("(p o) -> p o", o=1))
        scal_col = cst.tile([128, 1], F32)
        nc.scalar.dma_start(scal_col[:], scalc[:])

        # persistent SBUF
        xb = [big.tile([128, N], BF16, name=f"xb{b}", tag=f"xb{b}")
              for b in range(BL)]                      # 8 KB/part each
        sim_sb = big.tile([128, 4096], BF16)           # 8 KB/part (fold-2)
        mraw = big.tile([128, SL], BF16)               # raw memory bf16, 16 KB/part
        wb_t = big.tile([128, 4096], BF16)             # masked softmax W, 8 KB/part
        mtsb = big.tile([128, SL], BF16)               # normalized memory^T, 16 KB/part
        qacc = sml.tile([128, 32], F32)
        qT_all = sml.tile([128, B], F32)
        qTb = sml.tile([128, B], BF16)
        cands = sml.tile([128, 32], F32)
        cand_all = sml.tile([B, NCORES * 64], F32)
        mr_scr = sml.tile([B, NCORES * 64], F32)
        t16 = sml.tile([B, 16], F32)
        e16 = sml.tile([B, 16], F32)
        params = sml.tile([128, 4], F32)
        ssq = sml.tile([128, MT], F32)
        minv = sml.tile([128, MT], F32)
        proto_sb = sml.tile([B, D], F32)
        proto_loc = sml.tile([1, BL * D], F32)
        cwork = sml.tile([64, 8], F32)

        # ---- Phases A+B interleaved -------------------------------------
        with ExitStack() as pa:
            xstp = pa.enter_context(tc.tile_pool(name="xst", bufs=2))
            xt_sbp = pa.enter_context(tc.tile_pool(name="xt_sb", bufs=4))
            gelp = pa.enter_context(tc.tile_pool(name="gel", bufs=2))
            xt_ps = pa.enter_context(tc.tile_pool(name="xt_ps", bufs=4, space="PSUM"))
            ft_ps = pa.enter_context(tc.tile_pool(name="ft_ps", bufs=2, space="PSUM"))
            m_in = pa.enter_context(tc.tile_pool(name="m_in", bufs=2))
            sq_p = pa.enter_context(tc.tile_pool(name="sq", bufs=2))

            def emit_b_load(c):
                # memory chunk c (1024 rows): load + bf16 raw copy (GpSimd,
                # off everyone's critical path) + squared row norms (DVE
                # fused TT-square-reduce per 128-block).  Sqrt/normalize/
                # transpose are all deferred past the loop into the
                # q-AllGather shadow.
                mi = m_in.tile([128, 1024], F32, name="mi", tag="mi")
                nc.sync.dma_start(
                    mi[:].rearrange("p (t d) -> p t d", d=128),
                    ms[c * 1024:(c + 1) * 1024].rearrange("(t p) d -> p t d",
                                                          p=128),
                )
                nc.gpsimd.tensor_copy(mraw[:, c * 1024:(c + 1) * 1024], mi[:])
                sq = sq_p.tile([128, 1024], BF16, name="sq", tag="sq")
                nc.vector.tensor_tensor(sq[:], mi[:], mi[:], op=ALU.mult)
                nc.vector.tensor_reduce(
                    ssq[:, c * 8:c * 8 + 8],
                    sq[:].rearrange("p (t d) -> p t d", d=128),
                    axis=mybir.AxisListType.X, op=ALU.add,
                )

            def emit_a_batch(b):
                # batch b: load [128, 4096] f32 (p-outer: partition p owns
                # tokens p*32..p*32+31 -> contiguous 16KB DMA lines), cast to
                # bf16 (DVE, all 4 groups up front), then ALL transposes
                # before ALL feat matmuls so a matmul waiting on its PSUM
                # copy never blocks the next transpose in the PE FIFO.
                xstage = xstp.tile([128, N], F32)
                if b == 0:
                    # split the first load so compute starts ~4x earlier
                    for j in range(4):
                        nc.sync.dma_start(
                            xstage[:, j * 1024:(j + 1) * 1024].rearrange(
                                "p (t d) -> p t d", d=128),
                            xs[b].rearrange("(p t) d -> p t d", p=128)[
                                :, j * 8:(j + 1) * 8, :],
                        )
                else:
                    nc.sync.dma_start(
                        xstage[:].rearrange("p (t d) -> p t d", d=128),
                        xs[b].rearrange("(p t) d -> p t d", p=128),
                    )
                for j in range(4):
                    nc.vector.tensor_copy(
                        xb[b][:, j * 1024:(j + 1) * 1024],
                        xstage[:, j * 1024:(j + 1) * 1024],
                    )
                xsbs = []
                for j in range(4):          # 1024-col groups
                    base = j * 1024
                    xp = xt_ps.tile([128, 1024], BF16, name="xp", tag="xp")
                    for k in range(8):
                        nc.tensor.transpose(
                            xp[:, k * 128:(k + 1) * 128],
                            xb[b][:, base + k * 128:base + (k + 1) * 128],
                            identb[:],
                        )
                    xsb = xt_sbp.tile([128, 1024], BF16)
                    nc.vector.tensor_copy(xsb[:], xp[:])
                    xsbs.append(xsb)
                for j in range(4):
                    xsb = xsbs[j]
                    fp = ft_ps.tile([128, 1024], F32)
                    nc.tensor.matmul(fp[:, 0:512], wt_conv[:], xsb[:, 0:512],
                                     start=True, stop=True)
                    nc.tensor.matmul(fp[:, 512:1024], wt_conv[:],
                                     xsb[:, 512:1024], start=True, stop=True)
                    gl = gelp.tile([128, 1024], BF16, name="gl", tag="gl")
                    col = 4 * b + j
                    nc.scalar.activation(
                        gl[:], fp[:], AF.Gelu,
                        bias=bias_col[:], accum_out=qacc[:, col:col + 1],
                    )

            for b in range(BL):
                emit_b_load(b)
                emit_a_batch(b)

            qT = sml.tile([128, BL], F32)
            nc.vector.tensor_reduce(
                qT[:], qacc[:].rearrange("p (b g) -> p b g", g=4),
                axis=mybir.AxisListType.X, op=ALU.add,
            )
            nc.sync.dma_start(q_in[:], qT[:])

        nc.gpsimd.collective_compute(
            "AllGather", ALU.bypass, replica_groups=groups,
            ins=[q_in[:]], outs=[q_ag[:]],
        )

        # deferred memory normalization + transpose, in the q-AllGather
        # shadow (also keeps the PE busy so the HAM clock-gate stays open):
        # one batched sqrt (single act-table switch), then per chunk one
        # broadcast multiply from the resident bf16 mraw + 8 PE transposes.
        nc.scalar.activation(minv[:], ssq[:], AF.Sqrt, bias=zeros[:])
        nc.vector.reciprocal(minv[:], minv[:])
        with ExitStack() as pm:
            mn_p = pm.enter_context(tc.tile_pool(name="mn", bufs=2))
            mt_ps = pm.enter_context(tc.tile_pool(name="mt_ps", bufs=2,
                                                  space="PSUM"))
            for c in range(MC // 2):
                mn = mn_p.tile([128, 1024], BF16, name="mn", tag="mn")
                iv = minv[:, c * 8:c * 8 + 8]
                nc.vector.tensor_tensor(
                    mn[:].rearrange("p (t d) -> p t d", d=128),
                    mraw[:, c * 1024:(c + 1) * 1024].rearrange(
                        "p (t d) -> p t d", d=128),
                    iv.rearrange("p (t o) -> p t o", o=1).broadcast_to(
                        [128, 8, 128]),
                    op=ALU.mult,
                )
                mp = mt_ps.tile([128, 1024], BF16)
                for k in range(8):
                    nc.tensor.transpose(
                        mp[:, k * 128:(k + 1) * 128],
                        mn[:, k * 128:(k + 1) * 128], identb[:],
                    )
                nc.vector.tensor_copy(mtsb[:, c * 1024:(c + 1) * 1024], mp[:])

        nc.sync.dma_start(
            qT_all[:].rearrange("p (c b) -> p c b", c=NCORES),
            q_ag[:].rearrange("(c p) b -> p c b", p=128),
        )
        nc.vector.tensor_copy(qTb[:], qT_all[:])

        # cinv = 1/||q_b||, broadcast to both fold rows (b and 64+b)
        qsq = sml.tile([128, B], F32)
        nc.vector.tensor_tensor(qsq[:], qT_all[:], qT_all[:], op=ALU.mult)
        with tc.tile_pool(name="nrm_ps", bufs=1, space="PSUM") as nrmp:
            nrm = nrmp.tile([1, B], F32)
            nc.tensor.matmul(nrm[:], ones[:, 0:1], qsq[:], start=True, stop=True)
            nrow2 = sml.tile([1, 2 * B], F32)
            nc.scalar.activation(nrow2[0:1, 0:B], nrm[:], AF.Sqrt,
                                 bias=zeros[0:1, :])
            nc.vector.reciprocal(nrow2[0:1, 0:B], nrow2[0:1, 0:B])
            nc.vector.tensor_copy(nrow2[0:1, B:2 * B], nrow2[0:1, 0:B])
            ncol = nrmp.tile([128, 1], F32)
            nc.tensor.matmul(ncol[:], nrow2[:], ones[0:1, 0:1],
                             start=True, stop=True)
            nc.vector.tensor_copy(params[:, 0:1], ncol[:])

        # ---- sim matmuls (fold-2 into 128-part psum tiles) ---------------
        with tc.tile_pool(name="sim_ps", bufs=2, space="PSUM") as sim_ps:
            for cc in range(MC // 2):
                sp = sim_ps.tile([128, 512], F32)
                for half in range(2):
                    c = half * (MC // 2) + cc
                    nc.tensor.matmul(sp[half * 64:half * 64 + 64, :],
                                     qTb[:], mtsb[:, c * 512:(c + 1) * 512],
                                     start=True, stop=True)
                nc.vector.tensor_copy(sim_sb[:, cc * 512:(cc + 1) * 512], sp[:])

        for blk in range(4):
            nc.vector.max(
                cands[:, blk * 8:(blk + 1) * 8],
                sim_sb[:, blk * 1024:(blk + 1) * 1024],
            )
        nc.sync.dma_start(cand_in[:, 0:32], cands[0:64, :])
        nc.sync.dma_start(cand_in[:, 32:64], cands[64:128, :])

        nc.gpsimd.collective_compute(
            "AllGather", ALU.bypass, replica_groups=groups,
            ins=[cand_in[:]], outs=[cand_ag[:]],
        )

        # dense exp in the candidate-AllGather shadow, stabilized by the
        # per-fold local max; the global correction (a per-batch rescale)
        # is folded into the threshold mask after the merge.
        lmax = sml.tile([128, 1], F32)
        lbias = sml.tile([128, 1], F32)
        rsc = sml.tile([128, 1], F32)
        nc.vector.tensor_reduce(lmax[:], cands[:],
                                axis=mybir.AxisListType.X, op=ALU.max)
        nc.vector.tensor_tensor(lbias[:], lmax[:], params[:, 0:1], op=ALU.mult)
        nc.vector.tensor_scalar_mul(lbias[:], lbias[:], -1.0)
        for quar in range(4):
            qs = slice(quar * 1024, (quar + 1) * 1024)
            nc.scalar.activation(
                wb_t[:, qs], sim_sb[:, qs], AF.Exp,
                bias=lbias[:, 0:1], scale=params[:, 0:1],
            )
        nc.sync.dma_start(
            cand_all[:].rearrange("b (c j) -> b c j", c=NCORES),
            cand_ag[:].rearrange("(c b) j -> b c j", b=B),
        )

        # ---- merge: global top-16, softmax scalars -----------------------
        nc.vector.max(t16[:, 0:8], cand_all[:])
        nc.vector.match_replace(mr_scr[:], t16[:, 0:8], cand_all[:], NEG_BIG)
        nc.vector.max(t16[:, 8:16], mr_scr[:])

        nc.vector.tensor_tensor(cwork[:, 0:1], t16[:, 0:1], params[0:B, 0:1],
                                op=ALU.mult)
        nc.vector.tensor_scalar_mul(cwork[:, 1:2], cwork[:, 0:1], -1.0)
        nc.scalar.activation(e16[:], t16[:], AF.Exp,
                             bias=cwork[:, 1:2], scale=params[0:B, 0:1])
        nc.vector.tensor_reduce(cwork[:, 2:3], e16[:],
                                axis=mybir.AxisListType.X, op=ALU.add)
        nc.scalar.activation(cwork[:, 3:4], cwork[:, 2:3], AF.Ln,
                             bias=zeros[0:B, :])
        nc.vector.tensor_tensor(params[0:B, 1:2], cwork[:, 1:2], cwork[:, 3:4],
                                op=ALU.subtract)
        nc.vector.tensor_copy(params[0:B, 2:3], t16[:, 15:16])
        nc.sync.dma_start(params[64:128, 1:3], params[0:64, 1:3])
        # rescale = exp(params1 - lbias): converts locally-stabilized exps
        # into globally-normalized softmax weights.
        nc.vector.tensor_tensor(rsc[:], params[:, 1:2], lbias[:], op=ALU.subtract)
        nc.scalar.activation(rsc[:], rsc[:], AF.Exp, bias=zeros[:])

        if debug:
            dbg_t16 = nc.dram_tensor("dbg_t16", [B, 16], F32,
                                     kind="ExternalOutput")
            dbg_params = nc.dram_tensor("dbg_params", [128, 4], F32,
                                        kind="ExternalOutput")
            dbg_proto = nc.dram_tensor("dbg_proto", [B, D], F32,
                                       kind="ExternalOutput")
            nc.sync.dma_start(dbg_t16[:], t16[:])
            nc.sync.dma_start(dbg_params[:], params[:])
            nc.sync.dma_start(dbg_proto[:], proto_sb[:])

        # ---- Phase D: dense masked softmax W -> partial proto ------------
        with ExitStack() as pd:
            maskp = pd.enter_context(tc.tile_pool(name="mask", bufs=2))
            wt_sbp = pd.enter_context(tc.tile_pool(name="wt_sb", bufs=2))
            wt_psp = pd.enter_context(tc.tile_pool(name="wt_ps", bufs=2, space="PSUM"))
            pr_ps = pd.enter_context(tc.tile_pool(name="pr_ps", bufs=1, space="PSUM"))

            for quar in range(4):
                qs = slice(quar * 1024, (quar + 1) * 1024)
                mk = maskp.tile([128, 1024], BF16)
                nc.vector.tensor_scalar(
                    mk[:], sim_sb[:, qs], params[:, 2:3], rsc[:, 0:1],
                    op0=ALU.is_ge, op1=ALU.mult,
                )
                nc.vector.tensor_tensor(
                    wb_t[:, qs], wb_t[:, qs], mk[:], op=ALU.mult
                )

            pr = pr_ps.tile([64, 128], F32)
            for half in range(2):
                for k0 in range(0, 32, 8):
                    idh = identb[half * 64:half * 64 + 64,
                                 half * 64:half * 64 + 64]
                    wps = wt_psp.tile([128, 512], BF16)
                    for kk in range(8):
                        k = k0 + kk
                        nc.tensor.transpose(
                            wps[:, kk * 64:(kk + 1) * 64],
                            wb_t[half * 64:half * 64 + 64,
                                 k * 128:(k + 1) * 128],
                            idh,
                        )
                    wsb = wt_sbp.tile([128, 512], BF16)
                    nc.vector.tensor_copy(wsb[:], wps[:])
                    for kk in range(8):
                        t = half * 32 + k0 + kk
                        nc.tensor.matmul(
                            pr[:], wsb[:, kk * 64:(kk + 1) * 64],
                            mraw[:, t * 128:(t + 1) * 128],
                            start=(t == 0), stop=(t == MT - 1),
                        )
            nc.vector.tensor_copy(proto_sb[:], pr[:])
            nc.sync.dma_start(proto_in[:], proto_sb[:])

        nc.gpsimd.collective_compute(
            "ReduceScatter", ALU.add, replica_groups=groups,
            ins=[proto_in[:]], outs=[proto_rs[:]],
        )
        nc.sync.dma_start(proto_loc[:], proto_rs[:].rearrange("b d -> (b d)")
                          .rearrange("(o f) -> o f", o=1))

        # ---- Phase E: out = x + scale * proto broadcast (bf16) -----------
        with tc.tile_pool(name="bb_ps", bufs=2, space="PSUM") as bbp, \
             tc.tile_pool(name="bb_sb", bufs=2) as bbs:
            for b in range(BL):
                pb_ = bbp.tile([128, 128], F32)
                nc.tensor.matmul(pb_[:], ones[0:1, :],
                                 proto_loc[0:1, b * 128:(b + 1) * 128],
                                 start=True, stop=True)
                pbs = bbs.tile([128, 128], BF16)
                nc.vector.tensor_scalar(pbs[:], pb_[:], scal_col[:, 0:1],
                                        None, op0=ALU.mult)
                seg = xb[b][:].rearrange("p (t d) -> p t d", d=128)
                nc.vector.tensor_tensor(
                    seg, seg,
                    pbs[:].rearrange("p (o d) -> p o d", o=1).broadcast_to(
                        [128, N // 128, 128]
                    ),
                    op=ALU.add,
                )
                nc.sync.dma_start(
                    out_ext[b].rearrange("(p t) d -> p t d", p=128),
                    seg,
                )

    _hoist_waits(nc)
    return nc


_CACHED = {}


def kernel(x, conv_w, conv_b, memory, retrieval_scale):
    import ml_dtypes
    x = np.ascontiguousarray(np.asarray(x, dtype=np.float32))
    conv_wt = np.ascontiguousarray(
        np.asarray(conv_w, dtype=np.float32).T.astype(ml_dtypes.bfloat16))
    conv_b = np.ascontiguousarray(np.asarray(conv_b, dtype=np.float32))
    memory = np.ascontiguousarray(np.asarray(memory, dtype=np.float32))
    scalc = np.full((128, 1), np.asarray(retrieval_scale, dtype=np.float32),
                    dtype=np.float32)
    identb = np.eye(128, dtype=ml_dtypes.bfloat16)

    if "nc" not in _CACHED:
        _CACHED["nc"] = build_program()
    nc = _CACHED["nc"]

    in_maps = []
    for c in range(NCORES):
        in_maps.append({
            "xs": x[c * BL:(c + 1) * BL],
            "ms": memory[c * SL:(c + 1) * SL],
            "convwt": conv_wt,
            "convb": conv_b,
            "scalc": scalc,
            "identb": identb,
        })
    res = run_bass_kernel_spmd(nc, in_maps, list(range(NCORES)),
                               **_CACHED.get("run_kwargs", {}))
    _CACHED["last_result"] = res
    out = np.empty_like(x)
    for c in range(NCORES):
        out[c * BL:(c + 1) * BL] = np.asarray(res.results[c]["out"],
                                              dtype=np.float32)
    return out


# revision 29
# speedup vs baseline: 1.2486x; 1.0386x over previous
"""Trainium2 Bass kernel for nn_BPBookMemory (retrieval_knn).

Strategy (8 NeuronCores, SPMD):
  - x sharded by batch (8 per core); memory bank sharded 8-way (8192 rows/core).
  - Warmup collective triggered at t=0 (no input DMA) so the ~60us cold-start
    of the collectives subsystem overlaps Phase A instead of serializing.
  - Phase A: stream x (p-outer layout: each partition owns a contiguous
    32-token block -> 16KB DMA lines), cast to bf16 on GpSimd, PE-transpose,
    featT = gelu(W xT + b), accumulate q sums per batch on ACT (accum_out).
  - Phase B (interleaved with A in emission order so it overlaps): load
    memory shard, bf16 raw copy (GpSimd), row norms (ACT square+accum),
    normalize (DVE), PE-transpose -> mt tiles.
  - AllGather q -> all 64 query vectors everywhere.
  - sim[b, s_local] matmuls for all 64 batches; block-wise max8 gives 64
    candidate values per batch per core.
  - AllGather candidates -> identical merge on every core via max8 +
    match_replace + max8 -> global top-16 values, threshold, softmax scalars.
  - Dense masked softmax weights W = mask * exp(...) in bf16, PE-transpose,
    partial proto = W @ memory_shard; ReduceScatter(add).
  - out = x + retrieval_scale * proto, stored as bf16 (upcast to f32 on host;
    bf16 rounding of the output is ~0.2% rel, far under the 2e-2 gate).

Index-free top-k: only candidate VALUES travel; selection is by threshold
(sim >= 16th-largest), so no max_index / gather is ever needed.
"""

import os
import sys

for _p in ("/opt/trn_rl_repo", "/root/.axon_site/_ro/trn_rl_repo"):
    if os.path.isdir(_p) and _p not in sys.path:
        sys.path.append(_p)

import numpy as np
from contextlib import ExitStack

import concourse.bass as bass
import concourse.tile as tile
from concourse import mybir
from concourse.bass_utils import run_bass_kernel_spmd
from concourse.vector_clock import ScopedClock

F32 = mybir.dt.float32
BF16 = mybir.dt.bfloat16
AF = mybir.ActivationFunctionType
ALU = mybir.AluOpType

NCORES = 8
B, N, D, S = 64, 4096, 128, 65536
BL = B // NCORES          # 8 batches per core
SL = S // NCORES          # 8192 memory rows per core
MT = SL // 128            # 64 memory tiles per core
MC = SL // 512            # 16 memory chunks of 512
NEG_BIG = -1.0e30


# ---------------------------------------------------------------------------
# Walrus workaround: this container's neuronxcc rejects instructions carrying
# more than ~1 sync wait command (Drain/TPB_CTRL, LDWEIGHTS/S3_LW...).
# 1) Replace Tile's exit drain+barrier with EventSemaphore-carried waits.
# 2) Post-pass: hoist excess waits onto standalone EventSemaphore insts.
# ---------------------------------------------------------------------------

def _patched_drain_and_barrier(self, tick_clock, wait_clock):
    nc = self.nc
    carrier = nc.sync.add_instruction(
        mybir.InstEventSemaphore(name=f"I-{nc.next_id()}", ins=[], outs=[])
    )
    wait_clock.add_sem_waits(carrier.ins, ScopedClock({None: tick_clock.global_clock}))
    si = carrier.ins.sync_info
    waits = list(si.on_wait or [])
    if len(waits) > 1:
        si.on_wait = [waits[0]]
        for w in waits[1:]:
            extra = nc.sync.add_instruction(
                mybir.InstEventSemaphore(name=f"I-{nc.next_id()}", ins=[], outs=[])
            )
            extra.ins.sync_info = mybir.SyncInfo(on_wait=[w], on_update=[])
    for eng in nc.engines.values():
        eng.drain()
    nc.all_engine_barrier(sem_only=True)
    popped = nc._tile_sem_poison_stack.pop()
    assert popped is self._sem_poison
    nc.clear_and_free_semaphores(list(self.sems.allocated().values()))
    nc.all_engine_barrier(sem_only=True)


tile.TileContext._drain_and_barrier = _patched_drain_and_barrier

_hoist_ctr = [0]

import bass_rust as _bass_rust
_InstISA = _bass_rust.InstISA


def _hoist_waits(nc, max_keep=1):
    for f in nc.m.functions:
        for bb in f.blocks:
            insts = bb.instructions
            out = []
            changed = False
            for inst in insts:
                si = inst.sync_info
                waits = list(si.on_wait) if (si is not None and si.on_wait) else []
                if waits:
                    # Drain and raw-ISA instructions (e.g. TensorTensorReduce)
                    # cannot carry sem waits through this walrus build.
                    keep = (0 if (inst.opcode == "Drain"
                                  or isinstance(inst, _InstISA))
                            else max_keep)
                    kept, hoisted = [], []
                    for w in waits:
                        if len(kept) < keep and w.wait_mode == "sem-ge-imm":
                            kept.append(w)
                        else:
                            hoisted.append(w)
                    if hoisted:
                        for w in hoisted:
                            _hoist_ctr[0] += 1
                            ev = mybir.InstEventSemaphore(
                                name=f"I-hoistw-{_hoist_ctr[0]}", ins=[], outs=[]
                            )
                            ev.engine = inst.engine
                            ev.sync_info = mybir.SyncInfo(on_wait=[w], on_update=[])
                            out.append(ev)
                        si.on_wait = kept
                        changed = True
                out.append(inst)
            if changed:
                bb.instructions = out


# ---------------------------------------------------------------------------
# Kernel build
# ---------------------------------------------------------------------------

def build_program(debug=False):
    nc = bass.Bass(num_devices=NCORES)
    groups = [list(range(NCORES))]

    # raise Tile's stale SBUF cap (cayman has 208 KB usable per partition)
    import concourse.tile_utils as tile_utils
    if getattr(tile_utils, "max_sbuf_usage", 0) < 200 * 1024:
        tile_utils.max_sbuf_usage = 200 * 1024

    xs = nc.dram_tensor("xs", [BL, N, D], F32, kind="ExternalInput")
    ms = nc.dram_tensor("ms", [SL, D], F32, kind="ExternalInput")
    convwt = nc.dram_tensor("convwt", [D, D], BF16, kind="ExternalInput")
    convb = nc.dram_tensor("convb", [D], F32, kind="ExternalInput")
    scalc = nc.dram_tensor("scalc", [128, 1], F32, kind="ExternalInput")
    identb_in = nc.dram_tensor("identb", [128, 128], BF16, kind="ExternalInput")
    out_ext = nc.dram_tensor("out", [BL, N, D], BF16, kind="ExternalOutput")

    # collective bounce buffers
    warm_in = nc.dram_tensor("warm_in", [8, 4], F32)
    warm_out = nc.dram_tensor("warm_out", [8, 4], F32, addr_space="Shared")
    q_in = nc.dram_tensor("q_in", [128, BL], F32)
    q_ag = nc.dram_tensor("q_ag", [128 * NCORES, BL], F32, addr_space="Shared")
    cand_in = nc.dram_tensor("cand_in", [B, 64], F32)
    cand_ag = nc.dram_tensor("cand_ag", [B * NCORES, 64], F32, addr_space="Shared")
    proto_in = nc.dram_tensor("proto_in", [B, D], F32)
    proto_rs = nc.dram_tensor("proto_rs", [BL, D], F32)

    with tile.TileContext(nc) as tc, ExitStack() as top:
        # warmup collective FIRST: no input DMA (contents unused), so the
        # trigger has no dependencies and fires at t~0, absorbing the
        # collectives-subsystem cold start under Phase A.
        nc.gpsimd.collective_compute(
            "AllReduce", ALU.add, replica_groups=groups,
            ins=[warm_in[:]], outs=[warm_out[:]],
        )

        cst = top.enter_context(tc.tile_pool(name="cst", bufs=1))
        big = top.enter_context(tc.tile_pool(name="big", bufs=1))
        sml = top.enter_context(tc.tile_pool(name="sml", bufs=1))

        # constants on the scalar HWDGE ring so the sync ring starts x
        # loads immediately.  conv_w arrives pre-transposed in bf16 and the
        # scale pre-broadcast (host-side prep), so nothing downstream waits
        # on a setup compute chain.
        wt_conv = cst.tile([128, 128], BF16)
        nc.scalar.dma_start(wt_conv[:], convwt[:])
        identb = cst.tile([128, 128], BF16)
        nc.scalar.dma_start(identb[:], identb_in[:])
        ones = cst.tile([128, 128], F32)
        nc.gpsimd.memset(ones[:], 1.0)
        zeros = cst.tile([128, 1], F32)
        nc.gpsimd.memset(zeros[:], 0.0)
        bias_col = cst.tile([128, 1], F32)
        nc.scalar.dma_start(bias_col[:], convb[:].rearrange("(p o) -> p o", o=1))
        scal_col = cst.tile([128, 1], F32)
        nc.scalar.dma_start(scal_col[:], scalc[:])

        # persistent SBUF
        xb = [big.tile([128, N], BF16, name=f"xb{b}", tag=f"xb{b}")
              for b in range(BL)]                      # 8 KB/part each
        sim_sb = big.tile([128, 4096], BF16)           # 8 KB/part (fold-2)
        mraw = big.tile([128, SL], BF16)               # raw memory bf16, 16 KB/part
        wb_t = big.tile([128, 4096], BF16)             # masked softmax W, 8 KB/part
        mtsb = big.tile([128, SL], BF16)               # normalized memory^T, 16 KB/part
        qacc = sml.tile([128, 32], F32)
        qT_all = sml.tile([128, B], F32)
        qTb = sml.tile([128, B], BF16)
        cands = sml.tile([128, 32], F32)
        cand_all = sml.tile([B, NCORES * 64], F32)
        mr_scr = sml.tile([B, NCORES * 64], F32)
        t16 = sml.tile([B, 16], F32)
        e16 = sml.tile([B, 16], F32)
        params = sml.tile([128, 4], F32)
        ssq = sml.tile([128, MT], F32)
        minv = sml.tile([128, MT], F32)
        proto_sb = sml.tile([B, D], F32)
        proto_loc = sml.tile([1, BL * D], F32)
        cwork = sml.tile([64, 8], F32)

        # ---- Phases A+B interleaved -------------------------------------
        with ExitStack() as pa:
            xstp = pa.enter_context(tc.tile_pool(name="xst", bufs=2))
            xt_sbp = pa.enter_context(tc.tile_pool(name="xt_sb", bufs=4))
            gelp = pa.enter_context(tc.tile_pool(name="gel", bufs=2))
            xt_ps = pa.enter_context(tc.tile_pool(name="xt_ps", bufs=4, space="PSUM"))
            ft_ps = pa.enter_context(tc.tile_pool(name="ft_ps", bufs=2, space="PSUM"))
            m_in = pa.enter_context(tc.tile_pool(name="m_in", bufs=2))
            sq_p = pa.enter_context(tc.tile_pool(name="sq", bufs=2))

            def emit_b_load(c):
                # memory chunk c (1024 rows): load + bf16 raw copy (GpSimd,
                # off everyone's critical path) + squared row norms (DVE
                # fused TT-square-reduce per 128-block).  Sqrt/normalize/
                # transpose are all deferred past the loop into the
                # q-AllGather shadow.
                mi = m_in.tile([128, 1024], F32, name="mi", tag="mi")
                nc.sync.dma_start(
                    mi[:].rearrange("p (t d) -> p t d", d=128),
                    ms[c * 1024:(c + 1) * 1024].rearrange("(t p) d -> p t d",
                                                          p=128),
                )
                nc.gpsimd.tensor_copy(mraw[:, c * 1024:(c + 1) * 1024], mi[:])
                sq = sq_p.tile([128, 1024], BF16, name="sq", tag="sq")
                nc.scalar.activation(sq[:], mi[:], AF.Square, bias=zeros[:])
                nc.vector.tensor_reduce(
                    ssq[:, c * 8:c * 8 + 8],
                    sq[:].rearrange("p (t d) -> p t d", d=128),
                    axis=mybir.AxisListType.X, op=ALU.add,
                )

            def emit_a_batch(b):
                # batch b: load [128, 4096] f32 (p-outer: partition p owns
                # tokens p*32..p*32+31 -> contiguous 16KB DMA lines), cast to
                # bf16 (DVE, all 4 groups up front), then ALL transposes
                # before ALL feat matmuls so a matmul waiting on its PSUM
                # copy never blocks the next transpose in the PE FIFO.
                xstage = xstp.tile([128, N], F32)
                if b == 0:
                    # split the first load so compute starts ~4x earlier
                    for j in range(4):
                        nc.sync.dma_start(
                            xstage[:, j * 1024:(j + 1) * 1024].rearrange(
                                "p (t d) -> p t d", d=128),
                            xs[b].rearrange("(p t) d -> p t d", p=128)[
                                :, j * 8:(j + 1) * 8, :],
                        )
                else:
                    nc.sync.dma_start(
                        xstage[:].rearrange("p (t d) -> p t d", d=128),
                        xs[b].rearrange("(p t) d -> p t d", p=128),
                    )
                for j in range(4):
                    nc.vector.tensor_copy(
                        xb[b][:, j * 1024:(j + 1) * 1024],
                        xstage[:, j * 1024:(j + 1) * 1024],
                    )
                xsbs = []
                for j in range(4):          # 1024-col groups
                    base = j * 1024
                    xp = xt_ps.tile([128, 1024], BF16, name="xp", tag="xp")
                    for k in range(8):
                        nc.tensor.transpose(
                            xp[:, k * 128:(k + 1) * 128],
                            xb[b][:, base + k * 128:base + (k + 1) * 128],
                            identb[:],
                        )
                    xsb = xt_sbp.tile([128, 1024], BF16)
                    nc.vector.tensor_copy(xsb[:], xp[:])
                    xsbs.append(xsb)
                for j in range(4):
                    xsb = xsbs[j]
                    fp = ft_ps.tile([128, 1024], F32)
                    nc.tensor.matmul(fp[:, 0:512], wt_conv[:], xsb[:, 0:512],
                                     start=True, stop=True)
                    nc.tensor.matmul(fp[:, 512:1024], wt_conv[:],
                                     xsb[:, 512:1024], start=True, stop=True)
                    gl = gelp.tile([128, 1024], BF16, name="gl", tag="gl")
                    col = 4 * b + j
                    nc.scalar.activation(
                        gl[:], fp[:], AF.Gelu,
                        bias=bias_col[:], accum_out=qacc[:, col:col + 1],
                    )

            for b in range(BL):
                emit_b_load(b)
                emit_a_batch(b)

            qT = sml.tile([128, BL], F32)
            nc.vector.tensor_reduce(
                qT[:], qacc[:].rearrange("p (b g) -> p b g", g=4),
                axis=mybir.AxisListType.X, op=ALU.add,
            )
            nc.sync.dma_start(q_in[:], qT[:])

        nc.gpsimd.collective_compute(
            "AllGather", ALU.bypass, replica_groups=groups,
            ins=[q_in[:]], outs=[q_ag[:]],
        )

        # deferred memory normalization + transpose, in the q-AllGather
        # shadow (also keeps the PE busy so the HAM clock-gate stays open):
        # one batched sqrt (single act-table switch), then per chunk one
        # broadcast multiply from the resident bf16 mraw + 8 PE transposes.
        nc.scalar.activation(minv[:], ssq[:], AF.Sqrt, bias=zeros[:])
        nc.vector.reciprocal(minv[:], minv[:])
        with ExitStack() as pm:
            mn_p = pm.enter_context(tc.tile_pool(name="mn", bufs=2))
            mt_ps = pm.enter_context(tc.tile_pool(name="mt_ps", bufs=2,
                                                  space="PSUM"))
            for c in range(MC // 2):
                mn = mn_p.tile([128, 1024], BF16, name="mn", tag="mn")
                iv = minv[:, c * 8:c * 8 + 8]
                nc.vector.tensor_tensor(
                    mn[:].rearrange("p (t d) -> p t d", d=128),
                    mraw[:, c * 1024:(c + 1) * 1024].rearrange(
                        "p (t d) -> p t d", d=128),
                    iv.rearrange("p (t o) -> p t o", o=1).broadcast_to(
                        [128, 8, 128]),
                    op=ALU.mult,
                )
                mp = mt_ps.tile([128, 1024], BF16)
                for k in range(8):
                    nc.tensor.transpose(
                        mp[:, k * 128:(k + 1) * 128],
                        mn[:, k * 128:(k + 1) * 128], identb[:],
                    )
                nc.vector.tensor_copy(mtsb[:, c * 1024:(c + 1) * 1024], mp[:])

        nc.sync.dma_start(
            qT_all[:].rearrange("p (c b) -> p c b", c=NCORES),
            q_ag[:].rearrange("(c p) b -> p c b", p=128),
        )
        nc.vector.tensor_copy(qTb[:], qT_all[:])

        # cinv = 1/||q_b||, broadcast to both fold rows (b and 64+b)
        qsq = sml.tile([128, B], F32)
        nc.vector.tensor_tensor(qsq[:], qT_all[:], qT_all[:], op=ALU.mult)
        with tc.tile_pool(name="nrm_ps", bufs=1, space="PSUM") as nrmp:
            nrm = nrmp.tile([1, B], F32)
            nc.tensor.matmul(nrm[:], ones[:, 0:1], qsq[:], start=True, stop=True)
            nrow2 = sml.tile([1, 2 * B], F32)
            nc.scalar.activation(nrow2[0:1, 0:B], nrm[:], AF.Sqrt,
                                 bias=zeros[0:1, :])
            nc.vector.reciprocal(nrow2[0:1, 0:B], nrow2[0:1, 0:B])
            nc.vector.tensor_copy(nrow2[0:1, B:2 * B], nrow2[0:1, 0:B])
            ncol = nrmp.tile([128, 1], F32)
            nc.tensor.matmul(ncol[:], nrow2[:], ones[0:1, 0:1],
                             start=True, stop=True)
            nc.vector.tensor_copy(params[:, 0:1], ncol[:])

        # ---- sim matmuls (fold-2 into 128-part psum tiles) ---------------
        with tc.tile_pool(name="sim_ps", bufs=2, space="PSUM") as sim_ps:
            for cc in range(MC // 2):
                sp = sim_ps.tile([128, 512], F32)
                for half in range(2):
                    c = half * (MC // 2) + cc
                    nc.tensor.matmul(sp[half * 64:half * 64 + 64, :],
                                     qTb[:], mtsb[:, c * 512:(c + 1) * 512],
                                     start=True, stop=True)
                nc.vector.tensor_copy(sim_sb[:, cc * 512:(cc + 1) * 512], sp[:])

        for blk in range(4):
            nc.vector.max(
                cands[:, blk * 8:(blk + 1) * 8],
                sim_sb[:, blk * 1024:(blk + 1) * 1024],
            )
        nc.sync.dma_start(cand_in[:, 0:32], cands[0:64, :])
        nc.sync.dma_start(cand_in[:, 32:64], cands[64:128, :])

        nc.gpsimd.collective_compute(
            "AllGather", ALU.bypass, replica_groups=groups,
            ins=[cand_in[:]], outs=[cand_ag[:]],
        )

        # dense exp in the candidate-AllGather shadow, stabilized by the
        # per-fold local max; the global correction (a per-batch rescale)
        # is folded into the threshold mask after the merge.
        lmax = sml.tile([128, 1], F32)
        lbias = sml.tile([128, 1], F32)
        rsc = sml.tile([128, 1], F32)
        nc.vector.tensor_reduce(lmax[:], cands[:],
                                axis=mybir.AxisListType.X, op=ALU.max)
        nc.vector.tensor_tensor(lbias[:], lmax[:], params[:, 0:1], op=ALU.mult)
        nc.vector.tensor_scalar_mul(lbias[:], lbias[:], -1.0)
        for quar in range(4):
            qs = slice(quar * 1024, (quar + 1) * 1024)
            nc.scalar.activation(
                wb_t[:, qs], sim_sb[:, qs], AF.Exp,
                bias=lbias[:, 0:1], scale=params[:, 0:1],
            )
        nc.sync.dma_start(
            cand_all[:].rearrange("b (c j) -> b c j", c=NCORES),
            cand_ag[:].rearrange("(c b) j -> b c j", b=B),
        )

        # ---- merge: global top-16, softmax scalars -----------------------
        nc.vector.max(t16[:, 0:8], cand_all[:])
        nc.vector.match_replace(mr_scr[:], t16[:, 0:8], cand_all[:], NEG_BIG)
        nc.vector.max(t16[:, 8:16], mr_scr[:])

        nc.vector.tensor_tensor(cwork[:, 0:1], t16[:, 0:1], params[0:B, 0:1],
                                op=ALU.mult)
        nc.vector.tensor_scalar_mul(cwork[:, 1:2], cwork[:, 0:1], -1.0)
        nc.scalar.activation(e16[:], t16[:], AF.Exp,
                             bias=cwork[:, 1:2], scale=params[0:B, 0:1])
        nc.vector.tensor_reduce(cwork[:, 2:3], e16[:],
                                axis=mybir.AxisListType.X, op=ALU.add)
        nc.scalar.activation(cwork[:, 3:4], cwork[:, 2:3], AF.Ln,
                             bias=zeros[0:B, :])
        nc.vector.tensor_tensor(params[0:B, 1:2], cwork[:, 1:2], cwork[:, 3:4],
                                op=ALU.subtract)
        nc.vector.tensor_copy(params[0:B, 2:3], t16[:, 15:16])
        nc.sync.dma_start(params[64:128, 1:3], params[0:64, 1:3])
        # rescale = exp(params1 - lbias): converts locally-stabilized exps
        # into globally-normalized softmax weights.
        nc.vector.tensor_tensor(rsc[:], params[:, 1:2], lbias[:], op=ALU.subtract)
        nc.scalar.activation(rsc[:], rsc[:], AF.Exp, bias=zeros[:])

        if debug:
            dbg_t16 = nc.dram_tensor("dbg_t16", [B, 16], F32,
                                     kind="ExternalOutput")
            dbg_params = nc.dram_tensor("dbg_params", [128, 4], F32,
                                        kind="ExternalOutput")
            dbg_proto = nc.dram_tensor("dbg_proto", [B, D], F32,
                                       kind="ExternalOutput")
            nc.sync.dma_start(dbg_t16[:], t16[:])
            nc.sync.dma_start(dbg_params[:], params[:])
            nc.sync.dma_start(dbg_proto[:], proto_sb[:])

        # ---- Phase D: dense masked softmax W -> partial proto ------------
        with ExitStack() as pd:
            maskp = pd.enter_context(tc.tile_pool(name="mask", bufs=2))
            wt_sbp = pd.enter_context(tc.tile_pool(name="wt_sb", bufs=2))
            wt_psp = pd.enter_context(tc.tile_pool(name="wt_ps", bufs=2, space="PSUM"))
            pr_ps = pd.enter_context(tc.tile_pool(name="pr_ps", bufs=1, space="PSUM"))

            for quar in range(4):
                qs = slice(quar * 1024, (quar + 1) * 1024)
                mk = maskp.tile([128, 1024], BF16)
                nc.vector.tensor_scalar(
                    mk[:], sim_sb[:, qs], params[:, 2:3], rsc[:, 0:1],
                    op0=ALU.is_ge, op1=ALU.mult,
                )
                nc.vector.tensor_tensor(
                    wb_t[:, qs], wb_t[:, qs], mk[:], op=ALU.mult
                )

            pr = pr_ps.tile([64, 128], F32)
            for half in range(2):
                for k0 in range(0, 32, 8):
                    idh = identb[half * 64:half * 64 + 64,
                                 half * 64:half * 64 + 64]
                    wps = wt_psp.tile([128, 512], BF16)
                    for kk in range(8):
                        k = k0 + kk
                        nc.tensor.transpose(
                            wps[:, kk * 64:(kk + 1) * 64],
                            wb_t[half * 64:half * 64 + 64,
                                 k * 128:(k + 1) * 128],
                            idh,
                        )
                    wsb = wt_sbp.tile([128, 512], BF16)
                    nc.vector.tensor_copy(wsb[:], wps[:])
                    for kk in range(8):
                        t = half * 32 + k0 + kk
                        nc.tensor.matmul(
                            pr[:], wsb[:, kk * 64:(kk + 1) * 64],
                            mraw[:, t * 128:(t + 1) * 128],
                            start=(t == 0), stop=(t == MT - 1),
                        )
            nc.vector.tensor_copy(proto_sb[:], pr[:])
            nc.sync.dma_start(proto_in[:], proto_sb[:])

        nc.gpsimd.collective_compute(
            "ReduceScatter", ALU.add, replica_groups=groups,
            ins=[proto_in[:]], outs=[proto_rs[:]],
        )
        nc.sync.dma_start(proto_loc[:], proto_rs[:].rearrange("b d -> (b d)")
                          .rearrange("(o f) -> o f", o=1))

        # ---- Phase E: out = x + scale * proto broadcast (bf16) -----------
        with tc.tile_pool(name="bb_ps", bufs=2, space="PSUM") as bbp, \
             tc.tile_pool(name="bb_sb", bufs=2) as bbs:
            for b in range(BL):
                pb_ = bbp.tile([128, 128], F32)
                nc.tensor.matmul(pb_[:], ones[0:1, :],
                                 proto_loc[0:1, b * 128:(b + 1) * 128],
                                 start=True, stop=True)
                pbs = bbs.tile([128, 128], BF16)
                nc.vector.tensor_scalar(pbs[:], pb_[:], scal_col[:, 0:1],
                                        None, op0=ALU.mult)
                seg = xb[b][:].rearrange("p (t d) -> p t d", d=128)
                nc.vector.tensor_tensor(
                    seg, seg,
                    pbs[:].rearrange("p (o d) -> p o d", o=1).broadcast_to(
                        [128, N // 128, 128]
                    ),
                    op=ALU.add,
                )
                nc.sync.dma_start(
                    out_ext[b].rearrange("(p t) d -> p t d", p=128),
                    seg,
                )

    _hoist_waits(nc)
    return nc


_CACHED = {}


def kernel(x, conv_w, conv_b, memory, retrieval_scale):
    import ml_dtypes
    x = np.ascontiguousarray(np.asarray(x, dtype=np.float32))
    conv_wt = np.ascontiguousarray(
        np.asarray(conv_w, dtype=np.float32).T.astype(ml_dtypes.bfloat16))
    conv_b = np.ascontiguousarray(np.asarray(conv_b, dtype=np.float32))
    memory = np.ascontiguousarray(np.asarray(memory, dtype=np.float32))
    scalc = np.full((128, 1), np.asarray(retrieval_scale, dtype=np.float32),
                    dtype=np.float32)
    identb = np.eye(128, dtype=ml_dtypes.bfloat16)

    if "nc" not in _CACHED:
        _CACHED["nc"] = build_program()
    nc = _CACHED["nc"]

    in_maps = []
    for c in range(NCORES):
        in_maps.append({
            "xs": x[c * BL:(c + 1) * BL],
            "ms": memory[c * SL:(c + 1) * SL],
            "convwt": conv_wt,
            "convb": conv_b,
            "scalc": scalc,
            "identb": identb,
        })
    res = run_bass_kernel_spmd(nc, in_maps, list(range(NCORES)),
                               **_CACHED.get("run_kwargs", {}))
    _CACHED["last_result"] = res
    out = np.empty_like(x)
    for c in range(NCORES):
        out[c * BL:(c + 1) * BL] = np.asarray(res.results[c]["out"],
                                              dtype=np.float32)
    return out


# revision 40
# speedup vs baseline: 1.2866x; 1.0304x over previous
"""Trainium2 Bass kernel for nn_BPBookMemory (retrieval_knn).

Strategy (8 NeuronCores, SPMD):
  - x sharded by batch (8 per core); memory bank sharded 8-way (8192 rows/core).
  - Warmup collective triggered at t=0 (no input DMA) so the ~60us cold-start
    of the collectives subsystem overlaps Phase A instead of serializing.
  - Phase A: stream x (p-outer layout: each partition owns a contiguous
    32-token block -> 16KB DMA lines), cast to bf16 on GpSimd, PE-transpose,
    featT = gelu(W xT + b), accumulate q sums per batch on ACT (accum_out).
  - Phase B (interleaved with A in emission order so it overlaps): load
    memory shard, bf16 raw copy (GpSimd), row norms (ACT square+accum),
    normalize (DVE), PE-transpose -> mt tiles.
  - AllGather q -> all 64 query vectors everywhere.
  - sim[b, s_local] matmuls for all 64 batches; block-wise max8 gives 64
    candidate values per batch per core.
  - AllGather candidates -> identical merge on every core via max8 +
    match_replace + max8 -> global top-16 values, threshold, softmax scalars.
  - Dense masked softmax weights W = mask * exp(...) in bf16, PE-transpose,
    partial proto = W @ memory_shard; ReduceScatter(add).
  - out = x + retrieval_scale * proto, stored as bf16 (upcast to f32 on host;
    bf16 rounding of the output is ~0.2% rel, far under the 2e-2 gate).

Index-free top-k: only candidate VALUES travel; selection is by threshold
(sim >= 16th-largest), so no max_index / gather is ever needed.
"""

import os
import sys

for _p in ("/opt/trn_rl_repo", "/root/.axon_site/_ro/trn_rl_repo"):
    if os.path.isdir(_p) and _p not in sys.path:
        sys.path.append(_p)

import numpy as np
from contextlib import ExitStack

import concourse.bass as bass
import concourse.tile as tile
from concourse import mybir
from concourse.bass_utils import run_bass_kernel_spmd
from concourse.vector_clock import ScopedClock

F32 = mybir.dt.float32
BF16 = mybir.dt.bfloat16
AF = mybir.ActivationFunctionType
ALU = mybir.AluOpType

NCORES = 8
B, N, D, S = 64, 4096, 128, 65536
BL = B // NCORES          # 8 batches per core
SL = S // NCORES          # 8192 memory rows per core
MT = SL // 128            # 64 memory tiles per core
MC = SL // 512            # 16 memory chunks of 512
NEG_BIG = -1.0e30


# ---------------------------------------------------------------------------
# Walrus workaround: this container's neuronxcc rejects instructions carrying
# more than ~1 sync wait command (Drain/TPB_CTRL, LDWEIGHTS/S3_LW...).
# 1) Replace Tile's exit drain+barrier with EventSemaphore-carried waits.
# 2) Post-pass: hoist excess waits onto standalone EventSemaphore insts.
# ---------------------------------------------------------------------------

def _patched_drain_and_barrier(self, tick_clock, wait_clock):
    nc = self.nc
    carrier = nc.sync.add_instruction(
        mybir.InstEventSemaphore(name=f"I-{nc.next_id()}", ins=[], outs=[])
    )
    wait_clock.add_sem_waits(carrier.ins, ScopedClock({None: tick_clock.global_clock}))
    si = carrier.ins.sync_info
    waits = list(si.on_wait or [])
    if len(waits) > 1:
        si.on_wait = [waits[0]]
        for w in waits[1:]:
            extra = nc.sync.add_instruction(
                mybir.InstEventSemaphore(name=f"I-{nc.next_id()}", ins=[], outs=[])
            )
            extra.ins.sync_info = mybir.SyncInfo(on_wait=[w], on_update=[])
    for eng in nc.engines.values():
        eng.drain()
    nc.all_engine_barrier(sem_only=True)
    popped = nc._tile_sem_poison_stack.pop()
    assert popped is self._sem_poison
    nc.clear_and_free_semaphores(list(self.sems.allocated().values()))
    nc.all_engine_barrier(sem_only=True)


tile.TileContext._drain_and_barrier = _patched_drain_and_barrier

_hoist_ctr = [0]

import bass_rust as _bass_rust
_InstISA = _bass_rust.InstISA


def _hoist_waits(nc, max_keep=1):
    for f in nc.m.functions:
        for bb in f.blocks:
            insts = bb.instructions
            out = []
            changed = False
            for inst in insts:
                si = inst.sync_info
                waits = list(si.on_wait) if (si is not None and si.on_wait) else []
                if waits:
                    # Drain and raw-ISA instructions (e.g. TensorTensorReduce)
                    # cannot carry sem waits through this walrus build.
                    keep = (0 if (inst.opcode == "Drain"
                                  or isinstance(inst, _InstISA))
                            else max_keep)
                    kept, hoisted = [], []
                    for w in waits:
                        if len(kept) < keep and w.wait_mode == "sem-ge-imm":
                            kept.append(w)
                        else:
                            hoisted.append(w)
                    if hoisted:
                        for w in hoisted:
                            _hoist_ctr[0] += 1
                            ev = mybir.InstEventSemaphore(
                                name=f"I-hoistw-{_hoist_ctr[0]}", ins=[], outs=[]
                            )
                            ev.engine = inst.engine
                            ev.sync_info = mybir.SyncInfo(on_wait=[w], on_update=[])
                            out.append(ev)
                        si.on_wait = kept
                        changed = True
                out.append(inst)
            if changed:
                bb.instructions = out


# ---------------------------------------------------------------------------
# Kernel build
# ---------------------------------------------------------------------------

def build_program(debug=False):
    nc = bass.Bass(num_devices=NCORES)
    groups = [list(range(NCORES))]

    # raise Tile's stale SBUF cap (cayman has 208 KB usable per partition)
    import concourse.tile_utils as tile_utils
    if getattr(tile_utils, "max_sbuf_usage", 0) < 200 * 1024:
        tile_utils.max_sbuf_usage = 200 * 1024

    xs = nc.dram_tensor("xs", [BL, N, D], F32, kind="ExternalInput")
    ms = nc.dram_tensor("ms", [SL, D], F32, kind="ExternalInput")
    convwt = nc.dram_tensor("convwt", [D, D], BF16, kind="ExternalInput")
    convb = nc.dram_tensor("convb", [D], F32, kind="ExternalInput")
    scalc = nc.dram_tensor("scalc", [128, 1], F32, kind="ExternalInput")
    identb_in = nc.dram_tensor("identb", [128, 128], BF16, kind="ExternalInput")
    out_ext = nc.dram_tensor("out", [BL, N, D], BF16, kind="ExternalOutput")

    # collective bounce buffers
    warm_in = nc.dram_tensor("warm_in", [8, 4], F32)
    warm_out = nc.dram_tensor("warm_out", [8, 4], F32, addr_space="Shared")
    q_in = nc.dram_tensor("q_in", [128, BL], BF16)
    q_ag = nc.dram_tensor("q_ag", [128 * NCORES, BL], BF16,
                          addr_space="Shared")
    cand_in = nc.dram_tensor("cand_in", [B, 64], F32)
    cand_ag = nc.dram_tensor("cand_ag", [B * NCORES, 64], F32, addr_space="Shared")
    proto_in = nc.dram_tensor("proto_in", [B, D], F32)
    proto_rs = nc.dram_tensor("proto_rs", [BL, D], F32)

    with tile.TileContext(nc) as tc, ExitStack() as top:
        # warmup collective FIRST: no input DMA (contents unused), so the
        # trigger has no dependencies and fires at t~0, absorbing the
        # collectives-subsystem cold start under Phase A.
        nc.gpsimd.collective_compute(
            "AllReduce", ALU.add, replica_groups=groups,
            ins=[warm_in[:]], outs=[warm_out[:]],
        )

        cst = top.enter_context(tc.tile_pool(name="cst", bufs=1))
        big = top.enter_context(tc.tile_pool(name="big", bufs=1))
        sml = top.enter_context(tc.tile_pool(name="sml", bufs=1))

        # constants on the scalar HWDGE ring so the sync ring starts x
        # loads immediately.  conv_w arrives pre-transposed in bf16 and the
        # scale pre-broadcast (host-side prep), so nothing downstream waits
        # on a setup compute chain.
        wt_conv = cst.tile([128, 128], BF16)
        nc.scalar.dma_start(wt_conv[:], convwt[:])
        identb = cst.tile([128, 128], BF16)
        nc.scalar.dma_start(identb[:], identb_in[:])
        ones = cst.tile([128, 128], F32)
        nc.gpsimd.memset(ones[:], 1.0)
        zeros = cst.tile([128, 1], F32)
        nc.gpsimd.memset(zeros[:], 0.0)
        bias_col = cst.tile([128, 1], F32)
        nc.scalar.dma_start(bias_col[:], convb[:].rearrange("(p o) -> p o", o=1))
        scal_col = cst.tile([128, 1], F32)
        nc.scalar.dma_start(scal_col[:], scalc[:])

        # persistent SBUF
        xb = [big.tile([128, N], BF16, name=f"xb{b}", tag=f"xb{b}")
              for b in range(BL)]                      # 8 KB/part each
        sim_sb = big.tile([128, 4096], BF16)           # 8 KB/part (fold-2)
        mraw = big.tile([128, SL], BF16)               # raw memory bf16, 16 KB/part
        wb_t = big.tile([128, 4096], BF16)             # masked softmax W, 8 KB/part
        mtsb = big.tile([128, SL], BF16)               # normalized memory^T, 16 KB/part
        qacc = sml.tile([128, 32], F32)
        qTb = sml.tile([128, B], BF16)
        cinv_f = sml.tile([128, 1], F32)
        cands = sml.tile([128, 32], F32)
        cand_all = sml.tile([B, NCORES * 64], F32)
        mr_scr = sml.tile([B, NCORES * 64], F32)
        t16 = sml.tile([B, 16], F32)
        e16 = sml.tile([B, 16], F32)
        params = sml.tile([128, 4], F32)
        ssq = sml.tile([128, MT], F32)
        minv = sml.tile([128, MT], F32)
        proto_sb = sml.tile([B, D], F32)
        proto_loc = sml.tile([1, BL * D], F32)
        cwork = sml.tile([64, 8], F32)

        # ---- Phases A+B interleaved -------------------------------------
        with ExitStack() as pa:
            xstp = pa.enter_context(tc.tile_pool(name="xst", bufs=2))
            xt_sbp = pa.enter_context(tc.tile_pool(name="xt_sb", bufs=4))
            gelp = pa.enter_context(tc.tile_pool(name="gel", bufs=2))
            xt_ps = pa.enter_context(tc.tile_pool(name="xt_ps", bufs=4, space="PSUM"))
            ft_ps = pa.enter_context(tc.tile_pool(name="ft_ps", bufs=2, space="PSUM"))
            m_in = pa.enter_context(tc.tile_pool(name="m_in", bufs=2))
            sq_p = pa.enter_context(tc.tile_pool(name="sq", bufs=2))

            def emit_b_load(c):
                # memory chunk c (1024 rows): load + bf16 raw copy (GpSimd,
                # off everyone's critical path) + squared row norms (DVE
                # fused TT-square-reduce per 128-block).  Sqrt/normalize/
                # transpose are all deferred past the loop into the
                # q-AllGather shadow.
                mi = m_in.tile([128, 1024], F32, name="mi", tag="mi")
                nc.sync.dma_start(
                    mi[:].rearrange("p (t d) -> p t d", d=128),
                    ms[c * 1024:(c + 1) * 1024].rearrange("(t p) d -> p t d",
                                                          p=128),
                )
                nc.gpsimd.tensor_copy(mraw[:, c * 1024:(c + 1) * 1024], mi[:])
                sq = sq_p.tile([128, 1024], BF16, name="sq", tag="sq")
                nc.scalar.activation(sq[:], mi[:], AF.Square, bias=zeros[:])
                nc.vector.tensor_reduce(
                    ssq[:, c * 8:c * 8 + 8],
                    sq[:].rearrange("p (t d) -> p t d", d=128),
                    axis=mybir.AxisListType.X, op=ALU.add,
                )

            def emit_a_batch(b):
                # batch b: load [128, 4096] f32 (p-outer: partition p owns
                # tokens p*32..p*32+31 -> contiguous 16KB DMA lines), cast to
                # bf16 (DVE, all 4 groups up front), then ALL transposes
                # before ALL feat matmuls so a matmul waiting on its PSUM
                # copy never blocks the next transpose in the PE FIFO.
                xstage = xstp.tile([128, N], F32)
                nsplit = 4 if b == 0 else 2
                step = 4 // nsplit
                for j in range(nsplit):
                    nc.sync.dma_start(
                        xstage[:, j * step * 1024:(j + 1) * step * 1024]
                        .rearrange("p (t d) -> p t d", d=128),
                        xs[b].rearrange("(p t) d -> p t d", p=128)[
                            :, j * step * 8:(j + 1) * step * 8, :],
                    )
                for j in range(4):
                    nc.vector.tensor_copy(
                        xb[b][:, j * 1024:(j + 1) * 1024],
                        xstage[:, j * 1024:(j + 1) * 1024],
                    )
                xsbs = []
                for j in range(4):          # 1024-col groups
                    base = j * 1024
                    xp = xt_ps.tile([128, 1024], BF16, name="xp", tag="xp")
                    for k in range(8):
                        nc.tensor.transpose(
                            xp[:, k * 128:(k + 1) * 128],
                            xb[b][:, base + k * 128:base + (k + 1) * 128],
                            identb[:],
                        )
                    xsb = xt_sbp.tile([128, 1024], BF16)
                    nc.vector.tensor_copy(xsb[:], xp[:])
                    xsbs.append(xsb)
                for j in range(4):
                    xsb = xsbs[j]
                    fp = ft_ps.tile([128, 1024], F32)
                    nc.tensor.matmul(fp[:, 0:512], wt_conv[:], xsb[:, 0:512],
                                     start=True, stop=True)
                    nc.tensor.matmul(fp[:, 512:1024], wt_conv[:],
                                     xsb[:, 512:1024], start=True, stop=True)
                    gl = gelp.tile([128, 1024], BF16, name="gl", tag="gl")
                    col = 4 * b + j
                    nc.scalar.activation(
                        gl[:], fp[:], AF.Gelu,
                        bias=bias_col[:], accum_out=qacc[:, col:col + 1],
                    )
                nc.vector.tensor_reduce(
                    qT[:, b:b + 1],
                    qacc[:, 4 * b:4 * b + 4].rearrange("p (o g) -> p o g", o=1),
                    axis=mybir.AxisListType.X, op=ALU.add,
                )

            qT = sml.tile([128, BL], F32)
            for b in range(BL):
                emit_b_load(b)
                emit_a_batch(b)


        qstage = sml.tile([128, BL], BF16)
        nc.vector.tensor_copy(qstage[:], qT[:])
        nc.sync.dma_start(q_in[:], qstage[:])

        nc.gpsimd.collective_compute(
            "AllGather", ALU.bypass, replica_groups=groups,
            ins=[q_in[:]], outs=[q_ag[:]],
        )

        # deferred memory normalization + transpose, in the q-AllGather
        # shadow (also keeps the PE busy so the HAM clock-gate stays open):
        # one batched sqrt (single act-table switch), then per chunk one
        # broadcast multiply from the resident bf16 mraw + 8 PE transposes.
        nc.scalar.activation(minv[:], ssq[:], AF.Sqrt, bias=zeros[:])
        nc.vector.reciprocal(minv[:], minv[:])
        with ExitStack() as pm:
            mn_p = pm.enter_context(tc.tile_pool(name="mn", bufs=2))
            mt_ps = pm.enter_context(tc.tile_pool(name="mt_ps", bufs=2,
                                                  space="PSUM"))
            for c in range(MC // 2):
                mn = mn_p.tile([128, 1024], BF16, name="mn", tag="mn")
                iv = minv[:, c * 8:c * 8 + 8]
                nc.vector.tensor_tensor(
                    mn[:].rearrange("p (t d) -> p t d", d=128),
                    mraw[:, c * 1024:(c + 1) * 1024].rearrange(
                        "p (t d) -> p t d", d=128),
                    iv.rearrange("p (t o) -> p t o", o=1).broadcast_to(
                        [128, 8, 128]),
                    op=ALU.mult,
                )
                mp = mt_ps.tile([128, 1024], BF16)
                for k in range(8):
                    nc.tensor.transpose(
                        mp[:, k * 128:(k + 1) * 128],
                        mn[:, k * 128:(k + 1) * 128], identb[:],
                    )
                nc.vector.tensor_copy(mtsb[:, c * 1024:(c + 1) * 1024], mp[:])

        nc.sync.dma_start(
            qTb[:].rearrange("p (c b) -> p c b", c=NCORES),
            q_ag[:].rearrange("(c p) b -> p c b", p=128),
        )

        # ---- sim matmuls (fold-2 into 128-part psum tiles) ---------------
        with tc.tile_pool(name="sim_ps", bufs=2, space="PSUM") as sim_ps:
            for cc in range(MC // 2):
                sp = sim_ps.tile([128, 512], F32)
                for half in range(2):
                    c = half * (MC // 2) + cc
                    nc.tensor.matmul(sp[half * 64:half * 64 + 64, :],
                                     qTb[:], mtsb[:, c * 512:(c + 1) * 512],
                                     start=True, stop=True)
                nc.vector.tensor_copy(sim_sb[:, cc * 512:(cc + 1) * 512], sp[:])

        for blk in range(4):
            nc.vector.max(
                cands[:, blk * 8:(blk + 1) * 8],
                sim_sb[:, blk * 1024:(blk + 1) * 1024],
            )
        nc.sync.dma_start(cand_in[:, 0:32], cands[0:64, :])
        nc.sync.dma_start(cand_in[:, 32:64], cands[64:128, :])

        # cinv = 1/||q_b|| for all 64 batches, from the gathered bf16 q's,
        # overlapped with the candidate AllGather.
        qsqb = sml.tile([128, B], F32)
        nc.vector.tensor_tensor(qsqb[:], qTb[:], qTb[:], op=ALU.mult)
        with tc.tile_pool(name="nrm_ps", bufs=1, space="PSUM") as nrmp:
            nrmc = nrmp.tile([B, 1], F32)
            nc.tensor.matmul(nrmc[:], qsqb[:], ones[:, 0:1],
                             start=True, stop=True)
            nc.scalar.activation(cinv_f[0:B, 0:1], nrmc[:], AF.Sqrt,
                                 bias=zeros[0:B, :])
            nc.vector.reciprocal(cinv_f[0:B, 0:1], cinv_f[0:B, 0:1])
        nc.sync.dma_start(cinv_f[B:128, 0:1], cinv_f[0:B, 0:1])

        nc.gpsimd.collective_compute(
            "AllGather", ALU.bypass, replica_groups=groups,
            ins=[cand_in[:]], outs=[cand_ag[:]],
        )

        # dense exp in the candidate-AllGather shadow, stabilized by the
        # per-fold local max; the global correction (a per-batch rescale)
        # is folded into the threshold mask after the merge.
        lmax = sml.tile([128, 1], F32)
        lbias = sml.tile([128, 1], F32)
        rsc = sml.tile([128, 1], F32)
        nc.vector.tensor_reduce(lmax[:], cands[:],
                                axis=mybir.AxisListType.X, op=ALU.max)
        nc.vector.tensor_tensor(lbias[:], lmax[:], cinv_f[:, 0:1], op=ALU.mult)
        nc.vector.tensor_scalar_mul(lbias[:], lbias[:], -1.0)
        for quar in range(4):
            qs = slice(quar * 1024, (quar + 1) * 1024)
            nc.scalar.activation(
                wb_t[:, qs], sim_sb[:, qs], AF.Exp,
                bias=lbias[:, 0:1], scale=cinv_f[:, 0:1],
            )
        nc.sync.dma_start(
            cand_all[:].rearrange("b (c j) -> b c j", c=NCORES),
            cand_ag[:].rearrange("(c b) j -> b c j", b=B),
        )

        # ---- merge: global top-16, softmax scalars -----------------------
        nc.vector.max(t16[:, 0:8], cand_all[:])
        nc.vector.match_replace(mr_scr[:], t16[:, 0:8], cand_all[:], NEG_BIG)
        nc.vector.max(t16[:, 8:16], mr_scr[:])

        nc.vector.tensor_tensor(cwork[:, 0:1], t16[:, 0:1], cinv_f[0:B, 0:1],
                                op=ALU.mult)
        nc.vector.tensor_scalar_mul(cwork[:, 1:2], cwork[:, 0:1], -1.0)
        nc.scalar.activation(e16[:], t16[:], AF.Exp,
                             bias=cwork[:, 1:2], scale=cinv_f[0:B, 0:1])
        nc.vector.tensor_reduce(cwork[:, 2:3], e16[:],
                                axis=mybir.AxisListType.X, op=ALU.add)
        nc.scalar.activation(cwork[:, 3:4], cwork[:, 2:3], AF.Ln,
                             bias=zeros[0:B, :])
        nc.vector.tensor_tensor(params[0:B, 1:2], cwork[:, 1:2], cwork[:, 3:4],
                                op=ALU.subtract)
        nc.vector.tensor_copy(params[0:B, 2:3], t16[:, 15:16])
        nc.sync.dma_start(params[64:128, 1:3], params[0:64, 1:3])
        # rescale = exp(params1 - lbias): converts locally-stabilized exps
        # into globally-normalized softmax weights.
        nc.vector.tensor_tensor(rsc[:], params[:, 1:2], lbias[:], op=ALU.subtract)
        nc.scalar.activation(rsc[:], rsc[:], AF.Exp, bias=zeros[:])

        if debug:
            dbg_t16 = nc.dram_tensor("dbg_t16", [B, 16], F32,
                                     kind="ExternalOutput")
            dbg_params = nc.dram_tensor("dbg_params", [128, 4], F32,
                                        kind="ExternalOutput")
            dbg_proto = nc.dram_tensor("dbg_proto", [B, D], F32,
                                       kind="ExternalOutput")
            nc.sync.dma_start(dbg_t16[:], t16[:])
            nc.sync.dma_start(dbg_params[:], params[:])
            nc.sync.dma_start(dbg_proto[:], proto_sb[:])
            dbg_qtb = nc.dram_tensor("dbg_qtb", [128, B], BF16,
                                     kind="ExternalOutput")
            dbg_cinv = nc.dram_tensor("dbg_cinv", [128, 1], F32,
                                      kind="ExternalOutput")
            dbg_lb = nc.dram_tensor("dbg_lb", [128, 2], F32,
                                    kind="ExternalOutput")
            dbg_qag = nc.dram_tensor("dbg_qag", [128 * NCORES, BL + 1], BF16,
                                     kind="ExternalOutput")
            nc.sync.dma_start(dbg_qtb[:], qTb[:])
            nc.sync.dma_start(dbg_cinv[:], cinv_f[:])
            nc.sync.dma_start(dbg_lb[:, 0:1], lbias[:])
            nc.sync.dma_start(dbg_lb[:, 1:2], rsc[:])
            nc.sync.dma_start(dbg_qag[:], q_ag[:])

        # ---- Phase D: dense masked softmax W -> partial proto ------------
        with ExitStack() as pd:
            maskp = pd.enter_context(tc.tile_pool(name="mask", bufs=2))
            wt_sbp = pd.enter_context(tc.tile_pool(name="wt_sb", bufs=2))
            wt_psp = pd.enter_context(tc.tile_pool(name="wt_ps", bufs=2, space="PSUM"))
            pr_ps = pd.enter_context(tc.tile_pool(name="pr_ps", bufs=1, space="PSUM"))

            for quar in range(4):
                qs = slice(quar * 1024, (quar + 1) * 1024)
                mk = maskp.tile([128, 1024], BF16)
                nc.vector.tensor_scalar(
                    mk[:], sim_sb[:, qs], params[:, 2:3], rsc[:, 0:1],
                    op0=ALU.is_ge, op1=ALU.mult,
                )
                nc.vector.tensor_tensor(
                    wb_t[:, qs], wb_t[:, qs], mk[:], op=ALU.mult
                )

            pr = pr_ps.tile([64, 128], F32)
            for half in range(2):
                for k0 in range(0, 32, 8):
                    idh = identb[half * 64:half * 64 + 64,
                                 half * 64:half * 64 + 64]
                    wps = wt_psp.tile([128, 512], BF16)
                    for kk in range(8):
                        k = k0 + kk
                        nc.tensor.transpose(
                            wps[:, kk * 64:(kk + 1) * 64],
                            wb_t[half * 64:half * 64 + 64,
                                 k * 128:(k + 1) * 128],
                            idh,
                        )
                    wsb = wt_sbp.tile([128, 512], BF16)
                    nc.vector.tensor_copy(wsb[:], wps[:])
                    for kk in range(8):
                        t = half * 32 + k0 + kk
                        nc.tensor.matmul(
                            pr[:], wsb[:, kk * 64:(kk + 1) * 64],
                            mraw[:, t * 128:(t + 1) * 128],
                            start=(t == 0), stop=(t == MT - 1),
                        )
            nc.vector.tensor_copy(proto_sb[:], pr[:])
            nc.sync.dma_start(proto_in[:], proto_sb[:])

        nc.gpsimd.collective_compute(
            "ReduceScatter", ALU.add, replica_groups=groups,
            ins=[proto_in[:]], outs=[proto_rs[:]],
        )
        nc.sync.dma_start(proto_loc[:], proto_rs[:].rearrange("b d -> (b d)")
                          .rearrange("(o f) -> o f", o=1))

        # ---- Phase E: out = x + scale * proto broadcast (bf16) -----------
        with tc.tile_pool(name="bb_ps", bufs=2, space="PSUM") as bbp, \
             tc.tile_pool(name="bb_sb", bufs=2) as bbs:
            for b in range(BL):
                pb_ = bbp.tile([128, 128], F32)
                nc.tensor.matmul(pb_[:], ones[0:1, :],
                                 proto_loc[0:1, b * 128:(b + 1) * 128],
                                 start=True, stop=True)
                pbs = bbs.tile([128, 128], BF16)
                nc.vector.tensor_scalar(pbs[:], pb_[:], scal_col[:, 0:1],
                                        None, op0=ALU.mult)
                seg = xb[b][:].rearrange("p (t d) -> p t d", d=128)
                nc.vector.tensor_tensor(
                    seg, seg,
                    pbs[:].rearrange("p (o d) -> p o d", o=1).broadcast_to(
                        [128, N // 128, 128]
                    ),
                    op=ALU.add,
                )
                nc.sync.dma_start(
                    out_ext[b].rearrange("(p t) d -> p t d", p=128),
                    seg,
                )

    _hoist_waits(nc)
    return nc


_CACHED = {}


def kernel(x, conv_w, conv_b, memory, retrieval_scale):
    import ml_dtypes
    x = np.ascontiguousarray(np.asarray(x, dtype=np.float32))
    conv_wt = np.ascontiguousarray(
        np.asarray(conv_w, dtype=np.float32).T.astype(ml_dtypes.bfloat16))
    conv_b = np.ascontiguousarray(np.asarray(conv_b, dtype=np.float32))
    memory = np.ascontiguousarray(np.asarray(memory, dtype=np.float32))
    scalc = np.full((128, 1), np.asarray(retrieval_scale, dtype=np.float32),
                    dtype=np.float32)
    identb = np.eye(128, dtype=ml_dtypes.bfloat16)

    if "nc" not in _CACHED:
        _CACHED["nc"] = build_program()
    nc = _CACHED["nc"]

    in_maps = []
    for c in range(NCORES):
        in_maps.append({
            "xs": x[c * BL:(c + 1) * BL],
            "ms": memory[c * SL:(c + 1) * SL],
            "convwt": conv_wt,
            "convb": conv_b,
            "scalc": scalc,
            "identb": identb,
        })
    res = run_bass_kernel_spmd(nc, in_maps, list(range(NCORES)),
                               **_CACHED.get("run_kwargs", {}))
    _CACHED["last_result"] = res
    out = np.empty_like(x)
    for c in range(NCORES):
        out[c * BL:(c + 1) * BL] = np.asarray(res.results[c]["out"],
                                              dtype=np.float32)
    return out
